# revision 1
# baseline (speedup 1.0000x reference)
"""Trainium2 Bass kernel for AttentionAggregator (B=4, S=2048, H=1024, 16 heads).

Sharding: core = b*2 + hg  (b in 0..3 batch, hg in 0..1 head-group of 8 heads).

Math rewrite (pooling commutes with the output projection and with attn@v):
    pooled = mean_{s<L} (ctx[s]) @ Wo.T + bo,   ctx = attn @ v
    mean_{s<L} ctx = sum_k w_k * v[k],  w_k = (1/L) sum_{s<L} attn[s, k]
so the big attn@v matmul collapses to a weighted column-average of the
attention matrix plus a tiny matvec with v.  The final (B,1024)@Wo.T + bo is
done on host (8 MFLOP).

Key masking: the host zeroes x.T columns s >= L, so projected q/k/v for
invalid positions are exactly 0.  Invalid keys then contribute exp(0)=1 to the
softmax row-sum, which is corrected by subtracting (S - L).  Invalid v columns
are 0 so they add nothing to the output.  Invalid queries are killed by a
per-query scale qscale_s = [s<L]/L folded into the row weights r_s.

Per-core device program:
  1. projections qT/kT/vT = W.T-slices @ xT   (pair tiles [128=2*dk, 2048])
  2. per head h, per s-tile (128 queries):
       scoresT? no: scores[s,k] psum [128, 2048] = (qT st-slice).T @ kT
       E = exp(0.125*scores) -> SBUF, rowsum via ACT accum_out
       r = qscale * 1/(rowsum + negcnt)
       w[k] += r.T @ E   (M=128 replicated r, accumulated in PSUM over s-tiles)
  3. ctx_pooled[d] = sum_k vT[d,k]*w[k]  via one tensor_tensor_reduce per head
"""

import numpy as np

RSCALE = 4096.0  # keeps r = qscale/rowsum out of fp16 subnormal range
S = 2048          # sequence length
HDIM = 1024       # model dim
DK = 64           # head dim
NHC = 8           # heads per core
NPAIR = 4         # head pairs per core
KT = 8            # 128-row k-tiles in the HDIM contraction
NCHUNK = 4        # 512-wide chunks of S
NST = 16          # 128-row s-tiles
B = 4
NCORES = 8


def _build_program(debug=False):
    import concourse.mybir as mybir
    from concourse import bacc, tile

    f32 = mybir.dt.float32
    f16 = mybir.dt.float16
    nc = bacc.Bacc("TRN2", target_bir_lowering=False, debug=debug)

    xt_d = nc.dram_tensor("xt", [KT, 128, S], f16, kind="ExternalInput")
    wq_d = nc.dram_tensor("wq", [KT, 128, NPAIR, 128], f16, kind="ExternalInput")
    wk_d = nc.dram_tensor("wk", [KT, 128, NPAIR, 128], f16, kind="ExternalInput")
    wv_d = nc.dram_tensor("wv", [KT, 128, NPAIR, 128], f16, kind="ExternalInput")
    qs_d = nc.dram_tensor("qs", [128, NST], f32, kind="ExternalInput")
    ng_d = nc.dram_tensor("ng", [128, 1], f32, kind="ExternalInput")
    out_d = nc.dram_tensor("out", [128, NHC], f32, kind="ExternalOutput")

    with tile.TileContext(nc) as tc:
        with (
            tc.tile_pool(name="qkv", bufs=1) as qkv,
            tc.tile_pool(name="aux", bufs=1) as aux,
            tc.tile_pool(name="stat", bufs=8) as stat,
            tc.tile_pool(name="wpool", bufs=1) as wpool,
            tc.tile_pool(name="xpool", bufs=2) as xpool,
            tc.tile_pool(name="epool", bufs=5) as epool,
            tc.tile_pool(name="spool", bufs=2) as spool,
            tc.tile_pool(name="mmps", bufs=2, space="PSUM") as mmps,
            tc.tile_pool(name="wps_pool", bufs=1, space="PSUM") as wps_pool,
        ):
            qs_sb = aux.tile([128, NST], f32, name="qs_sb")
            ng_sb = aux.tile([128, 1], f32, name="ng_sb")
            ctx_sb = aux.tile([128, NHC], f32, name="ctx_sb")
            nc.vector.memset(ctx_sb[:], 0.0)
            nc.sync.dma_start(out=qs_sb[:], in_=qs_d[:])
            nc.sync.dma_start(out=ng_sb[:], in_=ng_d[:])

            qt = [qkv.tile([128, S], f16, name=f"qt{p}") for p in range(NPAIR)]
            kt = [qkv.tile([128, S], f16, name=f"kt{p}") for p in range(NPAIR)]
            vt = [qkv.tile([128, S], f32, name=f"vt{p}") for p in range(NPAIR)]

            # ---------------- projections ----------------
            wsb = {}
            for nm, dram in (("wq", wq_d), ("wk", wk_d), ("wv", wv_d)):
                t_ = wpool.tile([128, KT, NPAIR, 128], f16, name=f"{nm}_sb")
                for t in range(KT):
                    nc.sync.dma_start(out=t_[:, t], in_=dram[t])
                wsb[nm] = t_

            def proj_groups(pr):
                """Yield the 12 psum-group emitters for one head pair."""
                for c in range(NCHUNK):
                    xtile = xpool.tile([128, KT, 512], f16, name="xtile")
                    for t in range(KT):
                        nc.sync.dma_start(
                            out=xtile[:, t], in_=xt_d[t, :, c * 512:(c + 1) * 512]
                        )
                    for nm, dsts in (("wq", qt), ("wk", kt), ("wv", vt)):
                        def emit(nm=nm, dst=dsts[pr], xtile=xtile, c=c):
                            ps = mmps.tile([128, 512], f32, tag="mm", name="proj_ps")
                            for t in range(KT):
                                nc.tensor.matmul(
                                    ps[:],
                                    wsb[nm][:, t, pr],
                                    xtile[:, t],
                                    start=(t == 0),
                                    stop=(t == KT - 1),
                                )
                            nc.vector.tensor_copy(
                                dst[:, c * 512:(c + 1) * 512], ps[:]
                            )
                        yield emit

            # ---------------- attention ----------------
            def emit_head(h, fill=None):
                """One head; optionally interleave projection psum-groups
                (one per s-tile) so the PE always has queued work."""
                pr, lo = h // 2, (h % 2) * DK
                wps = wps_pool.tile([128, S], f32, name="wps")
                for st in range(NST):
                    e = epool.tile([128, S], f16, name="e")
                    rs = [None, None]
                    for half in range(2):
                        scps = mmps.tile([128, 1024], f32, tag="mm", name="sc_ps")
                        for cc in range(2):
                            c = half * 2 + cc
                            nc.tensor.matmul(
                                scps[:, cc * 512:(cc + 1) * 512],
                                qt[pr][lo:lo + DK, st * 128:(st + 1) * 128],
                                kt[pr][lo:lo + DK, c * 512:(c + 1) * 512],
                                start=True,
                                stop=True,
                            )
                        rs[half] = stat.tile([128, 1], f32, name=f"rs{half}")
                        nc.scalar.activation(
                            out=e[:, half * 1024:(half + 1) * 1024],
                            in_=scps[:],
                            func=mybir.ActivationFunctionType.Exp,
                            scale=0.125,
                            accum_out=rs[half][:],
                        )
                    rsum = stat.tile([128, 1], f32, name="rsum")
                    nc.vector.tensor_scalar(
                        out=rsum[:],
                        in0=rs[0][:],
                        scalar1=rs[1][:],
                        scalar2=ng_sb[:],
                        op0=mybir.AluOpType.add,
                        op1=mybir.AluOpType.add,
                    )
                    rinv = stat.tile([128, 1], f32, name="rinv")
                    nc.vector.reciprocal(rinv[:], rsum[:])
                    r = stat.tile([128, 1], f16, name="r")
                    nc.vector.tensor_mul(r[:], rinv[:], qs_sb[:, st:st + 1])
                    for c in range(NCHUNK):
                        nc.tensor.matmul(
                            wps[:, c * 512:(c + 1) * 512],
                            r.broadcast_to((128, 128)),
                            e[:, c * 512:(c + 1) * 512],
                            start=(st == 0),
                            stop=(st == NST - 1),
                        )
                    if fill is not None:
                        g = next(fill, None)
                        if g is not None:
                            g()
                scratch = spool.tile([128, S], f32, name="scratch")
                nc.vector.tensor_mul(
                    scratch[lo:lo + DK, :],
                    vt[pr][lo:lo + DK, :],
                    wps[lo:lo + DK, :],
                )
                nc.vector.reduce_sum(
                    ctx_sb[lo:lo + DK, h:h + 1],
                    scratch[lo:lo + DK, :],
                    axis=mybir.AxisListType.X,
                )

            # serial projections up front (empirically fastest: dense 8-MM
            # bursts keep the PE pipeline full), then ACT-paced attention.
            for pr in range(NPAIR):
                for g in proj_groups(pr):
                    g()
            for h in range(NHC):
                emit_head(h)

            nc.sync.dma_start(out=out_d[:], in_=ctx_sb[:])

    nc.compile()
    return nc


def _make_in_maps(x, L):
    x = np.asarray(x, dtype=np.float32)
    L = np.asarray(L)
    in_maps = []
    for core in range(NCORES):
        b, hg = core // 2, core % 2
        Lb = int(L[b])
        smask = (np.arange(S) < Lb).astype(np.float32)
        xT = x[b].T * smask[None, :]
        in_maps.append(
            {
                "xt": np.ascontiguousarray(xT.reshape(KT, 128, S), dtype=np.float16),
                "qs": np.ascontiguousarray(
                    (smask * np.float32(RSCALE / Lb)).reshape(NST, 128).T
                ),
                "ng": np.full((128, 1), -(S - Lb), dtype=np.float32),
            }
        )
    return in_maps


def _add_weights(in_maps, Wq, Wk, Wv):
    for core in range(NCORES):
        hg = core % 2
        for nm, W in (("wq", Wq), ("wk", Wk), ("wv", Wv)):
            ws = np.asarray(W, dtype=np.float32)[hg * 512:(hg + 1) * 512, :].T
            in_maps[core][nm] = np.ascontiguousarray(
                ws.reshape(KT, 128, NPAIR, 128), dtype=np.float16
            )
    return in_maps


def _postprocess(results, L, bv, Wo, bo):
    pooled = np.zeros((B, HDIM), dtype=np.float32)
    for core in range(NCORES):
        o = np.asarray(results[core]["out"])  # (128, NHC)
        b, hg = core // 2, core % 2
        for h in range(NHC):
            lo = (h % 2) * DK
            g = hg * NHC + h
            pooled[b, g * DK:(g + 1) * DK] = o[lo:lo + DK, h]
    pooled = pooled * np.float32(1.0 / RSCALE) + np.asarray(bv, dtype=np.float32)[None, :]
    out = pooled @ np.asarray(Wo, dtype=np.float32).T + np.asarray(bo, np.float32)
    return out.astype(np.float32)


_RUN_KWARGS = {}


def kernel(x, L, Wq, Wk, Wv, bv, Wo, bo):
    from concourse.bass_utils import run_bass_kernel_spmd

    nc = _build_program(debug=False)
    in_maps = _add_weights(_make_in_maps(x, L), Wq, Wk, Wv)
    res = run_bass_kernel_spmd(nc, in_maps, list(range(NCORES)), **_RUN_KWARGS)
    kernel.last_results = res
    return _postprocess(res.results, L, bv, Wo, bo)



# revision 2
# speedup vs baseline: 1.3762x; 1.3762x over previous
"""Trainium2 Bass kernel v2 for AttentionAggregator (B=4, S=2048, H=1024, 16 heads).

Sharding: core = b*2 + hg (b in 0..3 batch, hg head-group of 8 heads).

Transposed-attention design:
  scoresT[kpos, q] = kT_tile.T @ q        (per 128-kpos tile, per 1024-q half)
  ET = exp(scoresT/8)  -> SBUF f16        (one ACT instruction per k-tile, no accum)
  EV[d|den, q] += [vpos | ones].T @ ET    (PSUM-accumulated over 16 k-tiles)
The ones-column of the stationary yields the softmax denominator as a bonus
row, so no ACT accumulator reads and no per-tile DVE normalization chain.
Normalization, masking, mean-pool, bias and the Wo projection happen on host:
  pooled[d] = sum_q EV[d,q] / (L * (den_q - (S-L)))   for valid q.

Projections (q/k dims-major; v dims-major then PE-transposed to position-major
for the EV stationary) are interleaved into the ACT-bound attention stream.
PSUM: score ring 2x[128,1024] (4 banks) + EV [65,1024] (2) + proj [128,1024]
(2) = 8 banks exactly.
"""

import numpy as np

S = 2048
HDIM = 1024
DK = 64
NPAIR = 4
NKT = 16         # 128-row kpos tiles
B = 4
NCORES = 8


def _build_program(debug=False):
    import concourse.mybir as mybir
    from concourse import bacc, tile

    f32 = mybir.dt.float32
    f16 = mybir.dt.float16
    nc = bacc.Bacc("TRN2", target_bir_lowering=False, debug=debug)

    xt_d = nc.dram_tensor("xt", [8, 128, S], f16, kind="ExternalInput")
    wq_d = nc.dram_tensor("wq", [8, 128, NPAIR, 128], f16, kind="ExternalInput")
    wk_d = nc.dram_tensor("wk", [8, 128, NPAIR, 128], f16, kind="ExternalInput")
    wv_d = nc.dram_tensor("wv", [8, 128, NPAIR, 128], f16, kind="ExternalInput")
    id_d = nc.dram_tensor("ident", [128, 128], f16, kind="ExternalInput")
    ev_d = nc.dram_tensor("ev", [8, 2, 65, 1024], f32, kind="ExternalOutput")

    with tile.TileContext(nc) as tc:
        with (
            tc.tile_pool(name="const", bufs=1) as const,
            tc.tile_pool(name="qk", bufs=1) as qk,
            tc.tile_pool(name="vtp", bufs=2) as vtp,
            tc.tile_pool(name="vpp", bufs=1) as vpp,
            tc.tile_pool(name="etp", bufs=3) as etp,
            tc.tile_pool(name="evs", bufs=2) as evs,
            tc.tile_pool(name="ring", bufs=2, space="PSUM") as ring,
            tc.tile_pool(name="evp", bufs=1, space="PSUM") as evp,
            tc.tile_pool(name="pjp", bufs=1, space="PSUM") as pjp,
        ):
            xsb = const.tile([128, 8, S], f16, name="xsb")
            wsb = {
                nm: const.tile([128, 8, NPAIR, 128], f16, name=f"{nm}_sb")
                for nm in ("wq", "wk", "wv")
            }
            idsb = const.tile([128, 128], f16, name="idsb")

            # input DMAs: x by (half, t) so the first projection group can
            # start after the first 8 half-tiles
            for half in range(2):
                for t in range(8):
                    nc.sync.dma_start(
                        out=xsb[:, t, half * 1024:(half + 1) * 1024],
                        in_=xt_d[t, :, half * 1024:(half + 1) * 1024],
                    )
            for nm, dram in (("wq", wq_d), ("wk", wk_d), ("wv", wv_d)):
                for t in range(8):
                    nc.sync.dma_start(out=wsb[nm][:, t], in_=dram[t])
            nc.sync.dma_start(out=idsb[:], in_=id_d[:])

            qt = [qk.tile([128, S], f16, name=f"qt{p}") for p in range(NPAIR)]
            kt = [qk.tile([128, S], f16, name=f"kt{p}") for p in range(NPAIR)]
            vpos = [
                vpp.tile([128, NKT, 129], f16, name=f"vpos{p}") for p in range(NPAIR)
            ]
            vt_cur = {}

            # ---------- projection / transpose groups (fill stream) ----------
            def proj_qk(nm, pr, half, dst):
                def emit():
                    ps = pjp.tile([128, 1024], f32, tag="pj", name="proj_ps")
                    for t in range(8):
                        for cc in range(2):
                            nc.tensor.matmul(
                                ps[:, cc * 512:(cc + 1) * 512],
                                wsb[nm][:, t, pr],
                                xsb[:, t, half * 1024 + cc * 512:half * 1024 + (cc + 1) * 512],
                                start=(t == 0),
                                stop=(t == 7),
                            )
                    nc.vector.tensor_copy(
                        dst[:, half * 1024:(half + 1) * 1024], ps[:]
                    )
                return emit

            def proj_v(pr, half):
                def emit():
                    if half == 0:
                        vt_cur[pr] = vtp.tile([128, S], f16, name="vt")
                    ps = pjp.tile([128, 1024], f32, tag="pj", name="proj_ps")
                    for t in range(8):
                        for cc in range(2):
                            nc.tensor.matmul(
                                ps[:, cc * 512:(cc + 1) * 512],
                                wsb["wv"][:, t, pr],
                                xsb[:, t, half * 1024 + cc * 512:half * 1024 + (cc + 1) * 512],
                                start=(t == 0),
                                stop=(t == 7),
                            )
                    nc.vector.tensor_copy(
                        vt_cur[pr][:, half * 1024:(half + 1) * 1024], ps[:]
                    )
                return emit

            def transpose_v(pr, grp):
                def emit():
                    tps = pjp.tile([128, 8, 128], f16, tag="pj", name="tps")
                    vt = vt_cur[pr]
                    for k8 in range(8):
                        ktile = grp * 8 + k8
                        nc.tensor.transpose(
                            tps[:, k8],
                            vt[:, ktile * 128:(ktile + 1) * 128],
                            idsb[:],
                        )
                    dst = vpos[pr]
                    nc.vector.tensor_copy(
                        dst[:, grp * 8:(grp + 1) * 8, 0:64], tps[:, :, 0:64]
                    )
                    nc.vector.tensor_copy(
                        dst[:, grp * 8:(grp + 1) * 8, 65:129], tps[:, :, 64:128]
                    )
                    if grp == 1:
                        nc.vector.memset(dst[:, :, 64:65], 1.0)
                return emit

            def pair_items(pr):
                return [
                    proj_qk("wq", pr, 0, qt[pr]),
                    proj_qk("wk", pr, 0, kt[pr]),
                    proj_qk("wk", pr, 1, kt[pr]),
                    proj_v(pr, 0),
                    proj_v(pr, 1),
                    transpose_v(pr, 0),
                    transpose_v(pr, 1),
                    proj_qk("wq", pr, 1, qt[pr]),
                ]

            # lead-in: everything pair 0 needs
            for item in pair_items(0):
                item()
            fill = []
            for pr in range(1, NPAIR):
                fill.extend(pair_items(pr))
            fill_iter = iter(fill)

            # ---------- attention units ----------
            giter = [0]

            def unit(h, qh):
                pr, hh = h // 2, h % 2
                lo = hh * 64       # partition offset of this head in qt/kt
                lo2 = hh * 64      # column offset of this head in vpos
                evt = evp.tile([65, 1024], f32, tag="ev", name="evps")
                pend = []

                def ev_mm(ktile, et_t):
                    vsl = vpos[pr][:, ktile, lo2:lo2 + 65]
                    for cc in range(2):
                        nc.tensor.matmul(
                            evt[:, cc * 512:(cc + 1) * 512],
                            vsl,
                            et_t[:, cc * 512:(cc + 1) * 512],
                            start=(ktile == 0),
                            stop=(ktile == NKT - 1),
                        )

                for ktile in range(NKT):
                    ring_t = ring.tile([128, 1024], f32, tag="ring", name="sc_ps")
                    for cc in range(2):
                        nc.tensor.matmul(
                            ring_t[:, cc * 512:(cc + 1) * 512],
                            kt[pr][lo:lo + 64, ktile * 128:(ktile + 1) * 128],
                            qt[pr][lo:lo + 64,
                                   qh * 1024 + cc * 512:qh * 1024 + (cc + 1) * 512],
                            start=True,
                            stop=True,
                        )
                    et_t = etp.tile([128, 1024], f16, name="et")
                    nc.scalar.activation(
                        out=et_t[:],
                        in_=ring_t[:],
                        func=mybir.ActivationFunctionType.Exp,
                        scale=0.125,
                    )
                    pend.append((ktile, et_t))
                    if ktile >= 1:
                        ev_mm(*pend.pop(0))
                    giter[0] += 1
                    if giter[0] % 8 == 4:
                        g = next(fill_iter, None)
                        if g is not None:
                            g()
                ev_mm(*pend.pop(0))
                stage = evs.tile([65, 1024], f32, name="evstage")
                nc.vector.tensor_copy(stage[:], evt[:])
                nc.sync.dma_start(out=ev_d[h, qh], in_=stage[:])

            for h in range(8):
                for qh in range(2):
                    unit(h, qh)

    nc.compile()
    return nc


def _make_in_maps(x, L, Wq, Wk, Wv):
    x = np.asarray(x, dtype=np.float32)
    L = np.asarray(L)
    ident = np.eye(128, dtype=np.float16)
    in_maps = []
    for core in range(NCORES):
        b, hg = core // 2, core % 2
        Lb = int(L[b])
        smask = (np.arange(S) < Lb).astype(np.float32)
        xT = x[b].T * smask[None, :]
        m = {
            "xt": np.ascontiguousarray(xT.reshape(8, 128, S), dtype=np.float16),
            "ident": ident,
        }
        for nm, W in (("wq", Wq), ("wk", Wk), ("wv", Wv)):
            ws = np.asarray(W, dtype=np.float32)[hg * 512:(hg + 1) * 512, :].T
            m[nm] = np.ascontiguousarray(
                ws.reshape(8, 128, NPAIR, 128), dtype=np.float16
            )
        in_maps.append(m)
    return in_maps


def _postprocess(results, L, bv, Wo, bo):
    L = np.asarray(L)
    pooled = np.zeros((B, HDIM), dtype=np.float32)
    for core in range(NCORES):
        b, hg = core // 2, core % 2
        Lb = int(L[b])
        valid = np.arange(S) < Lb
        ev = np.asarray(results[core]["ev"])  # [8, 2, 65, 1024]
        for h in range(8):
            flat = np.concatenate([ev[h, 0], ev[h, 1]], axis=1)  # [65, 2048]
            if h % 2 == 0:
                dims, den = flat[0:64], flat[64]
            else:
                den, dims = flat[0], flat[1:65]
            den_true = den - np.float32(S - Lb)
            r = np.where(valid, 1.0 / (Lb * den_true), 0.0).astype(np.float32)
            g = hg * 8 + h
            pooled[b, g * 64:(g + 1) * 64] = dims @ r
    pooled = pooled + np.asarray(bv, dtype=np.float32)[None, :]
    out = pooled @ np.asarray(Wo, dtype=np.float32).T + np.asarray(bo, np.float32)
    return out.astype(np.float32)


_RUN_KWARGS = {}


def kernel(x, L, Wq, Wk, Wv, bv, Wo, bo):
    from concourse.bass_utils import run_bass_kernel_spmd

    nc = _build_program(debug=False)
    in_maps = _make_in_maps(x, L, Wq, Wk, Wv)
    res = run_bass_kernel_spmd(nc, in_maps, list(range(NCORES)), **_RUN_KWARGS)
    kernel.last_results = res
    return _postprocess(res.results, L, bv, Wo, bo)


# revision 4
# speedup vs baseline: 1.4917x; 1.0839x over previous
"""Trainium2 Bass kernel v4 for AttentionAggregator (B=4, S=2048, H=1024, 16 heads).

Sharding (L-balanced): core c handles heads (2c, 2c+1) of EVERY batch.
All cores run the identical program (SPMD); only the weight slices differ per
core. Per-batch loop bounds (key tiles, query chunks) are specialized to the
actual L values at build time, so padding work is skipped uniformly on all
cores.

Transposed-attention per (batch, head, query-half) unit:
  scoresT[kpos, q] = kT_tile.T @ q        (per 128-kpos tile)
  ET = exp(scoresT/8)  -> SBUF f16        (one ACT instruction per k-tile)
  EV[d|den, q] += [vpos | ones].T @ ET    (PSUM-accumulated over k-tiles)
The ones-column yields the softmax denominator as a bonus row. Host applies
normalization, masking, mean-pool, biases, and the Wo projection.

PSUM: score ring 2x[128,1024] (4 banks) + EV [65,1024] (2) + proj [128,1024]
(2) = 8 banks. Projections/transposes interleave into the ACT-bound stream.
"""

import numpy as np

S = 2048
HDIM = 1024
B = 4
NCORES = 8


def _bounds(L):
    ktn = [max(1, -(-int(l) // 128)) for l in L]          # key tiles of 128
    qch = []
    for l in L:
        l = int(l)
        qch.append([
            max(0, min(2, -(-min(l, 1024) // 512))),
            max(0, min(2, -(-(l - 1024) // 512))) if l > 1024 else 0,
        ])
    pch = [max(1, -(-(k * 128) // 512)) for k in ktn]      # k/v proj chunks
    qpch = [q[0] + q[1] for q in qch]                       # q proj chunks
    return ktn, qch, pch, qpch


def _build_program(L, debug=False):
    import concourse.mybir as mybir
    from concourse import bacc, tile

    f32 = mybir.dt.float32
    f16 = mybir.dt.float16
    nc = bacc.Bacc("TRN2", target_bir_lowering=False, debug=debug)

    KTN, QCH, PCH, QPCH = _bounds(L)

    xt_d = nc.dram_tensor("xt", [B, 8, 128, S], f16, kind="ExternalInput")
    wq_d = nc.dram_tensor("wq", [8, 128, 128], f16, kind="ExternalInput")
    wk_d = nc.dram_tensor("wk", [8, 128, 128], f16, kind="ExternalInput")
    wv_d = nc.dram_tensor("wv", [8, 128, 128], f16, kind="ExternalInput")
    id_d = nc.dram_tensor("ident", [128, 128], f16, kind="ExternalInput")
    ev_d = nc.dram_tensor("ev", [B, 2, 2, 65, 1024], f32, kind="ExternalOutput")

    with tile.TileContext(nc) as tc:
        with (
            tc.tile_pool(name="const", bufs=1) as const,
            tc.tile_pool(name="xp", bufs=2) as xp,
            tc.tile_pool(name="qk", bufs=1) as qk,
            tc.tile_pool(name="vtp", bufs=2) as vtp,
            tc.tile_pool(name="vpp", bufs=1) as vpp,
            tc.tile_pool(name="etp", bufs=3) as etp,
            tc.tile_pool(name="evs", bufs=2) as evs,
            tc.tile_pool(name="ring", bufs=2, space="PSUM") as ring,
            tc.tile_pool(name="evp", bufs=1, space="PSUM") as evp,
            tc.tile_pool(name="pjp", bufs=1, space="PSUM") as pjp,
        ):
            wsb = {
                nm: const.tile([128, 8, 128], f16, name=f"{nm}_sb")
                for nm in ("wq", "wk", "wv")
            }
            idsb = const.tile([128, 128], f16, name="idsb")
            for nm, dram in (("wq", wq_d), ("wk", wk_d), ("wv", wv_d)):
                for t in range(8):
                    nc.sync.dma_start(out=wsb[nm][:, t], in_=dram[t])
            nc.sync.dma_start(out=idsb[:], in_=id_d[:])

            # x tiles: double-buffered, loaded per batch in (half, t) order
            xsb = {}

            def dma_x(b):
                xsb[b] = xp.tile([128, 8, S], f16, name="xtile")
                nch = max(PCH[b], QPCH[b])  # 512-chunks of positions needed
                for half in range(2):
                    lo, hi = half * 1024, min(nch * 512, (half + 1) * 1024)
                    if hi <= lo:
                        continue
                    for t in range(8):
                        nc.sync.dma_start(
                            out=xsb[b][:, t, lo:hi], in_=xt_d[b, t, :, lo:hi]
                        )

            qt = [qk.tile([128, S], f16, name=f"qt{b}") for b in range(B)]
            kt = [qk.tile([128, S], f16, name=f"kt{b}") for b in range(B)]
            vpos = [qk.tile([128, 16, 129], f16, name=f"vpos{b}") for b in range(B)]
            vt_cur = {}

            # ---------- projection / transpose groups ----------
            def proj_grp(nm, b, half, dst, nch):
                """Project 512-chunks [2*half, min(nch,2*half+2)) of batch b."""
                ccs = [c for c in (0, 1) if half * 2 + c < nch]

                def emit():
                    if not ccs:
                        return
                    ps = pjp.tile([128, 1024], f32, tag="pj", name="proj_ps")
                    for t in range(8):
                        for cc in ccs:
                            o = half * 1024 + cc * 512
                            nc.tensor.matmul(
                                ps[:, cc * 512:(cc + 1) * 512],
                                wsb[nm][:, t],
                                xsb[b][:, t, o:o + 512],
                                start=(t == 0),
                                stop=(t == 7),
                            )
                    for cc in ccs:
                        o = half * 1024 + cc * 512
                        nc.vector.tensor_copy(
                            dst[:, o:o + 512], ps[:, cc * 512:(cc + 1) * 512]
                        )
                return emit

            def proj_v(b, half):
                nch = PCH[b]
                ccs = [c for c in (0, 1) if half * 2 + c < nch]

                def emit():
                    if half == 0:
                        vt_cur[b] = vtp.tile([128, S], f16, name="vt")
                    if not ccs:
                        return
                    ps = pjp.tile([128, 1024], f32, tag="pj", name="proj_ps")
                    for t in range(8):
                        for cc in ccs:
                            o = half * 1024 + cc * 512
                            nc.tensor.matmul(
                                ps[:, cc * 512:(cc + 1) * 512],
                                wsb["wv"][:, t],
                                xsb[b][:, t, o:o + 512],
                                start=(t == 0),
                                stop=(t == 7),
                            )
                    for cc in ccs:
                        o = half * 1024 + cc * 512
                        nc.vector.tensor_copy(
                            vt_cur[b][:, o:o + 512], ps[:, cc * 512:(cc + 1) * 512]
                        )
                return emit

            def transpose_v(b, grp):
                kts = [k for k in range(grp * 8, min((grp + 1) * 8, KTN[b]))]

                def emit():
                    if not kts:
                        return
                    tps = pjp.tile([128, 8, 128], f16, tag="pj", name="tps")
                    vt = vt_cur[b]
                    for i, ktile in enumerate(kts):
                        nc.tensor.transpose(
                            tps[:, i],
                            vt[:, ktile * 128:(ktile + 1) * 128],
                            idsb[:],
                        )
                    n = len(kts)
                    dst = vpos[b]
                    nc.vector.tensor_copy(
                        dst[:, kts[0]:kts[0] + n, 0:64], tps[:, 0:n, 0:64]
                    )
                    nc.vector.tensor_copy(
                        dst[:, kts[0]:kts[0] + n, 65:129], tps[:, 0:n, 64:128]
                    )
                    nc.vector.memset(dst[:, kts[0]:kts[0] + n, 64:65], 1.0)
                return emit

            def batch_items(b):
                return [
                    proj_grp("wq", b, 0, qt[b], QPCH[b]),
                    proj_grp("wk", b, 0, kt[b], PCH[b]),
                    proj_v(b, 0),
                    transpose_v(b, 0),
                    proj_grp("wk", b, 1, kt[b], PCH[b]),
                    proj_v(b, 1),
                    transpose_v(b, 1),
                    proj_grp("wq", b, 1, qt[b], QPCH[b]),
                ]

            # lead-in: batch 0 x + first-half projections; queue the rest
            dma_x(0)
            dma_x(1)
            lead = batch_items(0)
            for item in lead[:4]:
                item()
            fill = lead[4:]
            for b in range(1, B):
                fill.extend(batch_items(b))
            # x for batches 2,3 reuses the two x buffers; WAR deps order them
            dma_x(2)
            dma_x(3)
            fill_iter = iter(fill)
            nfill = [0]

            giter = [0]

            def pull_fill():
                g = next(fill_iter, None)
                if g is not None:
                    g()
                    nfill[0] += 1

            # ---------- attention units ----------
            def unit(b, hh, qh):
                nq = QCH[b][qh]
                if nq == 0:
                    return
                lo = hh * 64
                evt = evp.tile([65, 1024], f32, tag="ev", name="evps")
                pend = []
                ktn = KTN[b]

                def ev_mm(ktile, et_t):
                    vsl = vpos[b][:, ktile, lo:lo + 65]
                    for cc in range(nq):
                        nc.tensor.matmul(
                            evt[:, cc * 512:(cc + 1) * 512],
                            vsl,
                            et_t[:, cc * 512:(cc + 1) * 512],
                            start=(ktile == 0),
                            stop=(ktile == ktn - 1),
                        )

                for ktile in range(ktn):
                    ring_t = ring.tile([128, 1024], f32, tag="ring", name="sc_ps")
                    for cc in range(nq):
                        nc.tensor.matmul(
                            ring_t[:, cc * 512:(cc + 1) * 512],
                            kt[b][lo:lo + 64, ktile * 128:(ktile + 1) * 128],
                            qt[b][lo:lo + 64,
                                  qh * 1024 + cc * 512:qh * 1024 + (cc + 1) * 512],
                            start=True,
                            stop=True,
                        )
                    et_t = etp.tile([128, 1024], f16, name="et")
                    nc.scalar.activation(
                        out=et_t[:, 0:nq * 512],
                        in_=ring_t[:, 0:nq * 512],
                        func=mybir.ActivationFunctionType.Exp,
                        scale=0.125,
                    )
                    pend.append((ktile, et_t))
                    if ktile >= 1:
                        ev_mm(*pend.pop(0))
                    giter[0] += 1
                    cad = 2 if nfill[0] < 4 else 4
                    if giter[0] % cad == 0:
                        pull_fill()
                ev_mm(*pend.pop(0))
                stage = evs.tile([65, 1024], f32, name="evstage")
                nc.vector.tensor_copy(stage[:], evt[:])
                nc.sync.dma_start(out=ev_d[b, hh, qh], in_=stage[:])

            for b in range(B):
                for hh in range(2):
                    for qh in range(2):
                        unit(b, hh, qh)
            # drain any unpulled fill items (small L edge case)
            while True:
                g = next(fill_iter, None)
                if g is None:
                    break
                g()

    nc.compile()
    return nc


def _make_in_maps(x, L, Wq, Wk, Wv):
    x = np.asarray(x, dtype=np.float32)
    L = np.asarray(L)
    ident = np.eye(128, dtype=np.float16)
    xt = np.empty((B, 8, 128, S), dtype=np.float16)
    for b in range(B):
        smask = (np.arange(S) < int(L[b])).astype(np.float32)
        xt[b] = (x[b].T * smask[None, :]).reshape(8, 128, S).astype(np.float16)
    in_maps = []
    for core in range(NCORES):
        m = {"xt": xt, "ident": ident}
        for nm, W in (("wq", Wq), ("wk", Wk), ("wv", Wv)):
            ws = np.asarray(W, dtype=np.float32)[core * 128:(core + 1) * 128, :].T
            m[nm] = np.ascontiguousarray(ws.reshape(8, 128, 128), dtype=np.float16)
        in_maps.append(m)
    return in_maps


def _postprocess(results, L, bv, Wo, bo):
    L = np.asarray(L)
    KTN, QCH, _, _ = _bounds(L)
    pooled = np.zeros((B, HDIM), dtype=np.float32)
    for core in range(NCORES):
        ev = np.asarray(results[core]["ev"])  # [B, 2, 2, 65, 1024]
        for b in range(B):
            Lb = int(L[b])
            for hh in range(2):
                cols = []
                for qh in range(2):
                    nq = QCH[b][qh]
                    if nq:
                        cols.append(ev[b, hh, qh][:, :nq * 512])
                flat = np.concatenate(cols, axis=1)  # [65, ncols]
                ncols = flat.shape[1]
                if hh == 0:
                    dims, den = flat[0:64], flat[64]
                else:
                    den, dims = flat[0], flat[1:65]
                den_true = den - np.float32(KTN[b] * 128 - Lb)
                valid = np.arange(ncols) < Lb
                r = np.where(valid, 1.0 / (Lb * den_true), 0.0).astype(np.float32)
                g = core * 2 + hh
                pooled[b, g * 64:(g + 1) * 64] = dims @ r
    pooled = pooled + np.asarray(bv, dtype=np.float32)[None, :]
    out = pooled @ np.asarray(Wo, dtype=np.float32).T + np.asarray(bo, np.float32)
    return out.astype(np.float32)


_RUN_KWARGS = {}


def kernel(x, L, Wq, Wk, Wv, bv, Wo, bo):
    from concourse.bass_utils import run_bass_kernel_spmd

    nc = _build_program(np.asarray(L))
    in_maps = _make_in_maps(x, L, Wq, Wk, Wv)
    res = run_bass_kernel_spmd(nc, in_maps, list(range(NCORES)), **_RUN_KWARGS)
    kernel.last_results = res
    return _postprocess(res.results, L, bv, Wo, bo)


# revision 7
# speedup vs baseline: 1.5488x; 1.0383x over previous
"""Trainium2 Bass kernel v5 for AttentionAggregator (B=4, S=2048, H=1024, 16 heads).

Sharding (L-balanced): core c handles heads (2c, 2c+1) of EVERY batch; all
cores run the identical program, per-batch loop bounds specialized to L.

Transposed attention per (batch, head, query-half) unit:
  scoresT[kpos, q] = kT_tile.T @ q ; ET = exp(scoresT/8) -> f16 SBUF
  EV[d|den, q] += [vpos | ones].T @ ET   (PSUM, accumulated over k-tiles)
Ones-column gives softmax denominators. Host does normalization/pool/Wo.

v5 pipeline refinements over v4:
  - EV matmuls trail the score matmuls by TWO k-tiles so they never wait on
    the exp on the in-order PE queue.
  - Projections/transposes are interleaved as fine-grained per-t steps (2 MMs
    each) instead of 16-MM bursts, keeping the ACT stream fed.
  - Input DMAs spread across engine queues; lead-in starts after only the
    first half of batch 0.
"""

import numpy as np

S = 2048
HDIM = 1024
B = 4
NCORES = 8


def _bounds(L):
    ktn = [max(1, -(-int(l) // 128)) for l in L]
    qch = []
    for l in L:
        l = int(l)
        qch.append([
            max(0, min(2, -(-min(l, 1024) // 512))),
            max(0, min(2, -(-(l - 1024) // 512))) if l > 1024 else 0,
        ])
    pch = [max(1, -(-(k * 128) // 512)) for k in ktn]
    qpch = [q[0] + q[1] for q in qch]
    return ktn, qch, pch, qpch


def _build_program(L, debug=False):
    import concourse.mybir as mybir
    from concourse import bacc, tile

    f32 = mybir.dt.float32
    f16 = mybir.dt.float16
    nc = bacc.Bacc("TRN2", target_bir_lowering=False, debug=debug)

    KTN, QCH, PCH, QPCH = _bounds(L)

    xt_d = nc.dram_tensor("xt", [B, 8, 128, S], f16, kind="ExternalInput")
    wq_d = nc.dram_tensor("wq", [8, 128, 128], f16, kind="ExternalInput")
    wk_d = nc.dram_tensor("wk", [8, 128, 128], f16, kind="ExternalInput")
    wv_d = nc.dram_tensor("wv", [8, 128, 128], f16, kind="ExternalInput")
    id_d = nc.dram_tensor("ident", [128, 128], f16, kind="ExternalInput")
    ev_d = nc.dram_tensor("ev", [B, 2, 2, 65, 1024], f32, kind="ExternalOutput")

    with tile.TileContext(nc) as tc:
        with (
            tc.tile_pool(name="const", bufs=1) as const,
            tc.tile_pool(name="xp", bufs=2) as xp,
            tc.tile_pool(name="qk", bufs=1) as qk,
            tc.tile_pool(name="vtp", bufs=2) as vtp,
            tc.tile_pool(name="etp", bufs=4) as etp,
            tc.tile_pool(name="evs", bufs=2) as evs,
            tc.tile_pool(name="ring", bufs=2, space="PSUM") as ring,
            tc.tile_pool(name="evp", bufs=1, space="PSUM") as evp,
            tc.tile_pool(name="pjp", bufs=1, space="PSUM") as pjp,
        ):
            wsb = {
                nm: const.tile([128, 8, 128], f16, name=f"{nm}_sb")
                for nm in ("wq", "wk", "wv")
            }
            idsb = const.tile([128, 128], f16, name="idsb")

            xsb = {}

            def dma_x(b, halves=(0, 1)):
                if b not in xsb:
                    xsb[b] = xp.tile([128, 8, S], f16, name="xtile")
                nch = max(PCH[b], QPCH[b])
                engs = [nc.sync, nc.gpsimd] if b < 2 else [nc.sync, nc.sync]
                for half in halves:
                    lo, hi = half * 1024, min(nch * 512, (half + 1) * 1024)
                    if hi <= lo:
                        continue
                    for t in range(8):
                        engs[t % 2].dma_start(
                            out=xsb[b][:, t, lo:hi], in_=xt_d[b, t, :, lo:hi]
                        )

            # batch-0 first half + weights first so the lead-in starts fast
            dma_x(0, halves=(0,))
            for t in range(8):
                nc.scalar.dma_start(out=wsb["wq"][:, t], in_=wq_d[t])
            for t in range(8):
                nc.scalar.dma_start(out=wsb["wk"][:, t], in_=wk_d[t])
            for t in range(8):
                nc.scalar.dma_start(out=wsb["wv"][:, t], in_=wv_d[t])
            nc.scalar.dma_start(out=idsb[:], in_=id_d[:])
            dma_x(0, halves=(1,))
            dma_x(1)

            qt = [qk.tile([128, S], f16, name=f"qt{b}") for b in range(B)]
            kt = [qk.tile([128, S], f16, name=f"kt{b}") for b in range(B)]
            vpos = [qk.tile([128, 16, 129], f16, name=f"vpos{b}") for b in range(B)]
            vt_cur = {}

            # ---------- fine-grained projection / transpose steps ----------
            def proj_steps(nm, b, half, dst_fn):
                """Yield per-t matmul steps + a final copy step for one
                1024-column half of a projection."""
                nch = PCH[b] if nm != "wq" else QPCH[b]
                ccs = [c for c in (0, 1) if half * 2 + c < nch]
                if not ccs:
                    return
                cell = {}

                def step_t(t):
                    def emit():
                        if t == 0:
                            cell["ps"] = pjp.tile(
                                [128, 1024], f32, tag="pj", name="proj_ps"
                            )
                        ps = cell["ps"]
                        for cc in ccs:
                            o = half * 1024 + cc * 512
                            nc.tensor.matmul(
                                ps[:, cc * 512:(cc + 1) * 512],
                                wsb[nm][:, t],
                                xsb[b][:, t, o:o + 512],
                                start=(t == 0),
                                stop=(t == 7),
                            )
                    return emit

                for t in range(8):
                    yield step_t(t)

                def copy_step():
                    ps = cell["ps"]
                    dst = dst_fn()
                    for cc in ccs:
                        o = half * 1024 + cc * 512
                        nc.vector.tensor_copy(
                            dst[:, o:o + 512], ps[:, cc * 512:(cc + 1) * 512]
                        )
                yield copy_step

            def v_alloc(b):
                def emit():
                    vt_cur[b] = vtp.tile([128, S], f16, name="vt")
                return emit

            def transpose_steps(b, grp):
                kts = list(range(grp * 8, min((grp + 1) * 8, KTN[b])))
                if not kts:
                    return
                cell = {}

                def tstep(sub):
                    def emit():
                        if sub == 0:
                            cell["tps"] = pjp.tile(
                                [128, 8, 128], f16, tag="pj", name="tps"
                            )
                        tps = cell["tps"]
                        for i in range(sub * 4, min((sub + 1) * 4, len(kts))):
                            nc.tensor.transpose(
                                tps[:, i],
                                vt_cur[b][:, kts[i] * 128:(kts[i] + 1) * 128],
                                idsb[:],
                            )
                    return emit

                yield tstep(0)
                if len(kts) > 4:
                    yield tstep(1)

                def copy_step():
                    tps = cell["tps"]
                    n = len(kts)
                    dst = vpos[b]
                    nc.vector.tensor_copy(
                        dst[:, kts[0]:kts[0] + n, 0:64], tps[:, 0:n, 0:64]
                    )
                    nc.vector.tensor_copy(
                        dst[:, kts[0]:kts[0] + n, 65:129], tps[:, 0:n, 64:128]
                    )
                    nc.vector.memset(dst[:, kts[0]:kts[0] + n, 64:65], 1.0)
                yield copy_step

            def batch_steps(b, first):
                """first: the part needed before attention on b can start
                (half 0 of q/k/v + transpose grp 0); rest comes via fill."""
                steps = []
                if first:
                    steps.extend(proj_steps("wq", b, 0, lambda b=b: qt[b]))
                    steps.extend(proj_steps("wk", b, 0, lambda b=b: kt[b]))
                    steps.append(v_alloc(b))
                    steps.extend(proj_steps("wv", b, 0, lambda b=b: vt_cur[b]))
                    steps.extend(transpose_steps(b, 0))
                else:
                    steps.extend(proj_steps("wk", b, 1, lambda b=b: kt[b]))
                    steps.extend(proj_steps("wv", b, 1, lambda b=b: vt_cur[b]))
                    steps.extend(transpose_steps(b, 1))
                    steps.extend(proj_steps("wq", b, 1, lambda b=b: qt[b]))
                return steps

            # lead-in: batch 0 first-half projections, emitted contiguously
            for s in batch_steps(0, True):
                s()
            fill = batch_steps(0, False)
            for b in range(1, B):
                fill.extend(batch_steps(b, True))
                fill.extend(batch_steps(b, False))
            dma_x(2)
            dma_x(3)
            fill_iter = iter(fill)
            done = [False]

            def pull_fill(n):
                for _ in range(n):
                    g = next(fill_iter, None)
                    if g is None:
                        done[0] = True
                        return
                    g()

            giter = [0]
            pend = []  # deferred PE/DVE work, drained ~2 iterations later

            def tick():
                giter[0] += 1
                while len(pend) > 2:
                    pend.pop(0)()
                # ~1.3 fill steps per iteration (3 early to beat kt-tile 8)
                pull_fill(3 if giter[0] <= 12 else (2 if giter[0] % 3 == 0 else 1))

            # ---------- attention units ----------
            def unit(b, hh, qh):
                nq = QCH[b][qh]
                if nq == 0:
                    return
                lo = hh * 64
                evt = evp.tile([65, 1024], f32, tag="ev", name="evps")
                ktn = KTN[b]

                def ev_mm(ktile, et_t):
                    def emit():
                        vsl = vpos[b][:, ktile, lo:lo + 65]
                        for cc in range(nq):
                            nc.tensor.matmul(
                                evt[:, cc * 512:(cc + 1) * 512],
                                vsl,
                                et_t[:, cc * 512:(cc + 1) * 512],
                                start=(ktile == 0),
                                stop=(ktile == ktn - 1),
                            )
                    return emit

                def writeout():
                    def emit():
                        stage = evs.tile([65, 1024], f32, name="evstage")
                        nc.vector.tensor_copy(stage[:], evt[:])
                        nc.gpsimd.dma_start(out=ev_d[b, hh, qh], in_=stage[:])
                    return emit

                for ktile in range(ktn):
                    ring_t = ring.tile([128, 1024], f32, tag="ring", name="sc_ps")
                    for cc in range(nq):
                        nc.tensor.matmul(
                            ring_t[:, cc * 512:(cc + 1) * 512],
                            kt[b][lo:lo + 64, ktile * 128:(ktile + 1) * 128],
                            qt[b][lo:lo + 64,
                                  qh * 1024 + cc * 512:qh * 1024 + (cc + 1) * 512],
                            start=True,
                            stop=True,
                        )
                    et_t = etp.tile([128, 1024], f16, name="et")
                    nc.scalar.activation(
                        out=et_t[:, 0:nq * 512],
                        in_=ring_t[:, 0:nq * 512],
                        func=mybir.ActivationFunctionType.Exp,
                        scale=0.125,
                    )
                    pend.append(ev_mm(ktile, et_t))
                    tick()
                pend.append(writeout())

            for b in range(B):
                for hh in range(2):
                    for qh in range(2):
                        unit(b, hh, qh)
            for p in pend:
                p()
            while not done[0]:
                pull_fill(1)

    nc.compile()
    return nc


def _make_in_maps(x, L, Wq, Wk, Wv):
    x = np.asarray(x, dtype=np.float32)
    L = np.asarray(L)
    ident = np.eye(128, dtype=np.float16)
    xt = np.empty((B, 8, 128, S), dtype=np.float16)
    for b in range(B):
        smask = (np.arange(S) < int(L[b])).astype(np.float32)
        xt[b] = (x[b].T * smask[None, :]).reshape(8, 128, S).astype(np.float16)
    in_maps = []
    for core in range(NCORES):
        m = {"xt": xt, "ident": ident}
        for nm, W in (("wq", Wq), ("wk", Wk), ("wv", Wv)):
            ws = np.asarray(W, dtype=np.float32)[core * 128:(core + 1) * 128, :].T
            m[nm] = np.ascontiguousarray(ws.reshape(8, 128, 128), dtype=np.float16)
        in_maps.append(m)
    return in_maps


def _postprocess(results, L, bv, Wo, bo):
    L = np.asarray(L)
    KTN, QCH, _, _ = _bounds(L)
    pooled = np.zeros((B, HDIM), dtype=np.float32)
    for core in range(NCORES):
        ev = np.asarray(results[core]["ev"])  # [B, 2, 2, 65, 1024]
        for b in range(B):
            Lb = int(L[b])
            for hh in range(2):
                cols = []
                for qh in range(2):
                    nq = QCH[b][qh]
                    if nq:
                        cols.append(ev[b, hh, qh][:, :nq * 512])
                flat = np.concatenate(cols, axis=1)
                ncols = flat.shape[1]
                if hh == 0:
                    dims, den = flat[0:64], flat[64]
                else:
                    den, dims = flat[0], flat[1:65]
                den_true = den - np.float32(KTN[b] * 128 - Lb)
                valid = np.arange(ncols) < Lb
                r = np.where(valid, 1.0 / (Lb * den_true), 0.0).astype(np.float32)
                g = core * 2 + hh
                pooled[b, g * 64:(g + 1) * 64] = dims @ r
    pooled = pooled + np.asarray(bv, dtype=np.float32)[None, :]
    out = pooled @ np.asarray(Wo, dtype=np.float32).T + np.asarray(bo, np.float32)
    return out.astype(np.float32)


_RUN_KWARGS = {}


def kernel(x, L, Wq, Wk, Wv, bv, Wo, bo):
    from concourse.bass_utils import run_bass_kernel_spmd

    nc = _build_program(np.asarray(L))
    in_maps = _make_in_maps(x, L, Wq, Wk, Wv)
    res = run_bass_kernel_spmd(nc, in_maps, list(range(NCORES)), **_RUN_KWARGS)
    kernel.last_results = res
    return _postprocess(res.results, L, bv, Wo, bo)


# revision 9
# speedup vs baseline: 1.5714x; 1.0146x over previous
"""Trainium2 Bass kernel v5 for AttentionAggregator (B=4, S=2048, H=1024, 16 heads).

Sharding (L-balanced): core c handles heads (2c, 2c+1) of EVERY batch; all
cores run the identical program, per-batch loop bounds specialized to L.

Transposed attention per (batch, head, query-half) unit:
  scoresT[kpos, q] = kT_tile.T @ q ; ET = exp(scoresT/8) -> f16 SBUF
  EV[d|den, q] += [vpos | ones].T @ ET   (PSUM, accumulated over k-tiles)
Ones-column gives softmax denominators. Host does normalization/pool/Wo.

v5 pipeline refinements over v4:
  - EV matmuls trail the score matmuls by TWO k-tiles so they never wait on
    the exp on the in-order PE queue.
  - Projections/transposes are interleaved as fine-grained per-t steps (2 MMs
    each) instead of 16-MM bursts, keeping the ACT stream fed.
  - Input DMAs spread across engine queues; lead-in starts after only the
    first half of batch 0.
"""

import numpy as np

S = 2048
HDIM = 1024
B = 4
NCORES = 8


def _bounds(L):
    ktn = [max(1, -(-int(l) // 128)) for l in L]
    qch = []
    for l in L:
        l = int(l)
        qch.append([
            max(0, min(2, -(-min(l, 1024) // 512))),
            max(0, min(2, -(-(l - 1024) // 512))) if l > 1024 else 0,
        ])
    pch = [max(1, -(-(k * 128) // 512)) for k in ktn]
    qpch = [q[0] + q[1] for q in qch]
    return ktn, qch, pch, qpch


def _build_program(L, debug=False):
    import concourse.mybir as mybir
    from concourse import bacc, tile

    f32 = mybir.dt.float32
    f16 = mybir.dt.float16
    nc = bacc.Bacc("TRN2", target_bir_lowering=False, debug=debug)

    KTN, QCH, PCH, QPCH = _bounds(L)

    xt_d = nc.dram_tensor("xt", [B, 2, 8, 128, 1024], f16, kind="ExternalInput")
    wq_d = nc.dram_tensor("wq", [8, 128, 128], f16, kind="ExternalInput")
    wk_d = nc.dram_tensor("wk", [8, 128, 128], f16, kind="ExternalInput")
    wv_d = nc.dram_tensor("wv", [8, 128, 128], f16, kind="ExternalInput")
    id_d = nc.dram_tensor("ident", [128, 128], f16, kind="ExternalInput")
    ev_d = nc.dram_tensor("ev", [B, 2, 2, 65, 1024], f32, kind="ExternalOutput")

    with tile.TileContext(nc) as tc:
        with (
            tc.tile_pool(name="const", bufs=1) as const,
            tc.tile_pool(name="xp", bufs=2) as xp,
            tc.tile_pool(name="qk", bufs=1) as qk,
            tc.tile_pool(name="vtp", bufs=2) as vtp,
            tc.tile_pool(name="etp", bufs=6) as etp,
            tc.tile_pool(name="evs", bufs=2) as evs,
            tc.tile_pool(name="ring", bufs=2, space="PSUM") as ring,
            tc.tile_pool(name="evp", bufs=1, space="PSUM") as evp,
            tc.tile_pool(name="pjp", bufs=1, space="PSUM") as pjp,
        ):
            wsb = {
                nm: const.tile([128, 8, 128], f16, name=f"{nm}_sb")
                for nm in ("wq", "wk", "wv")
            }
            idsb = const.tile([128, 128], f16, name="idsb")

            xsb = {}

            def dma_x(b, halves=(0, 1)):
                if b not in xsb:
                    xsb[b] = xp.tile([128, 8, S], f16, name="xtile")
                nch = max(PCH[b], QPCH[b])
                engs = [nc.sync, nc.gpsimd] if b < 2 else [nc.sync, nc.sync]
                for half in halves:
                    lo, hi = half * 1024, min(nch * 512, (half + 1) * 1024)
                    if hi <= lo:
                        continue
                    for t in range(8):
                        engs[t % 2].dma_start(
                            out=xsb[b][:, t, lo:hi],
                            in_=xt_d[b, half, t, :, 0:hi - lo],
                        )

            # batch-0 first half + ident first so the lead-in starts fast
            nc.scalar.dma_start(out=idsb[:], in_=id_d[:])
            dma_x(0, halves=(0,))
            # warm the PE clock gate throughout the x DMA window: each burst
            # waits for one x shard to land, keeping the activity monitor busy
            wu = pjp.tile([128, 1024], f32, tag="pj", name="warmup")
            for t in range(8):
                for i in range(8):
                    nc.tensor.matmul(
                        wu[:, 0:128], idsb[:], xsb[0][:, t, 0:128],
                        start=True, stop=True,
                    )
            for t in range(8):
                nc.scalar.dma_start(out=wsb["wq"][:, t], in_=wq_d[t])
            for t in range(8):
                nc.scalar.dma_start(out=wsb["wk"][:, t], in_=wk_d[t])
            for t in range(8):
                nc.scalar.dma_start(out=wsb["wv"][:, t], in_=wv_d[t])
            dma_x(0, halves=(1,))
            dma_x(1)

            qt = [qk.tile([128, S], f16, name=f"qt{b}") for b in range(B)]
            kt = [qk.tile([128, S], f16, name=f"kt{b}") for b in range(B)]
            vpos = [qk.tile([128, 16, 129], f16, name=f"vpos{b}") for b in range(B)]
            vt_cur = {}

            # ---------- fine-grained projection / transpose steps ----------
            def proj_steps(nm, b, half, dst_fn):
                """Yield per-t matmul steps + a final copy step for one
                1024-column half of a projection."""
                nch = PCH[b] if nm != "wq" else QPCH[b]
                ccs = [c for c in (0, 1) if half * 2 + c < nch]
                if not ccs:
                    return
                cell = {}

                def step_t(t):
                    def emit():
                        if t == 0:
                            cell["ps"] = pjp.tile(
                                [128, 1024], f32, tag="pj", name="proj_ps"
                            )
                        ps = cell["ps"]
                        for cc in ccs:
                            o = half * 1024 + cc * 512
                            nc.tensor.matmul(
                                ps[:, cc * 512:(cc + 1) * 512],
                                wsb[nm][:, t],
                                xsb[b][:, t, o:o + 512],
                                start=(t == 0),
                                stop=(t == 7),
                            )
                    return emit

                for t in range(8):
                    yield step_t(t)

                def copy_step():
                    ps = cell["ps"]
                    dst = dst_fn()
                    for cc in ccs:
                        o = half * 1024 + cc * 512
                        nc.vector.tensor_copy(
                            dst[:, o:o + 512], ps[:, cc * 512:(cc + 1) * 512]
                        )
                yield copy_step

            def v_alloc(b):
                def emit():
                    vt_cur[b] = vtp.tile([128, S], f16, name="vt")
                return emit

            def transpose_steps(b, grp):
                kts = list(range(grp * 8, min((grp + 1) * 8, KTN[b])))
                if not kts:
                    return
                cell = {}

                def tstep(sub):
                    def emit():
                        if sub == 0:
                            cell["tps"] = pjp.tile(
                                [128, 8, 128], f16, tag="pj", name="tps"
                            )
                        tps = cell["tps"]
                        for i in range(sub * 4, min((sub + 1) * 4, len(kts))):
                            nc.tensor.transpose(
                                tps[:, i],
                                vt_cur[b][:, kts[i] * 128:(kts[i] + 1) * 128],
                                idsb[:],
                            )
                    return emit

                yield tstep(0)
                if len(kts) > 4:
                    yield tstep(1)

                def copy_step():
                    tps = cell["tps"]
                    n = len(kts)
                    dst = vpos[b]
                    nc.vector.tensor_copy(
                        dst[:, kts[0]:kts[0] + n, 0:64], tps[:, 0:n, 0:64]
                    )
                    nc.vector.tensor_copy(
                        dst[:, kts[0]:kts[0] + n, 65:129], tps[:, 0:n, 64:128]
                    )
                    nc.vector.memset(dst[:, kts[0]:kts[0] + n, 64:65], 1.0)
                yield copy_step

            def batch_steps(b, first):
                """first: the part needed before attention on b can start
                (half 0 of q/k/v + transpose grp 0); rest comes via fill."""
                steps = []
                if first:
                    steps.extend(proj_steps("wq", b, 0, lambda b=b: qt[b]))
                    steps.extend(proj_steps("wk", b, 0, lambda b=b: kt[b]))
                    steps.append(v_alloc(b))
                    steps.extend(proj_steps("wv", b, 0, lambda b=b: vt_cur[b]))
                    steps.extend(transpose_steps(b, 0))
                else:
                    steps.extend(proj_steps("wk", b, 1, lambda b=b: kt[b]))
                    steps.extend(proj_steps("wv", b, 1, lambda b=b: vt_cur[b]))
                    steps.extend(transpose_steps(b, 1))
                    steps.extend(proj_steps("wq", b, 1, lambda b=b: qt[b]))
                return steps

            # lead-in: only q/k first halves of batch 0 emitted contiguously;
            # v+transposes land in the first two iterations via the fill
            lead = []
            lead.extend(proj_steps("wq", 0, 0, lambda: qt[0]))
            lead.extend(proj_steps("wk", 0, 0, lambda: kt[0]))
            for s in lead:
                s()

            def due_range(steps, lo, hi):
                n = max(1, len(steps))
                return [
                    (lo + (hi - lo) * i // n, s) for i, s in enumerate(steps)
                ]

            fill = []
            vpart = [v_alloc(0)]
            vpart.extend(proj_steps("wv", 0, 0, lambda: vt_cur[0]))
            vpart.extend(transpose_steps(0, 0))
            fill += due_range(vpart, 0, 1)           # needed by EV(kt0) ~iter 2
            rest0 = batch_steps(0, False)
            fill += due_range(rest0, 2, 14)          # k/v h1 by kt 8, q h1 by 16
            # per-batch unit-start iterations
            ustart = [0]
            for b in range(B):
                ustart.append(
                    ustart[-1]
                    + KTN[b] * sum(2 for q in range(2) if QCH[b][q] > 0)
                )
            for b in range(1, B):
                steps = batch_steps(b, True) + batch_steps(b, False)
                lo = ustart[b - 1] + (18 if b == 1 else 2)
                fill += due_range(steps, lo, ustart[b] - 10)
            dma_x(2)
            dma_x(3)
            fill.sort(key=lambda p: p[0])
            fidx = [0]
            done = [False]

            def pull_due(force=False):
                while fidx[0] < len(fill) and (
                    force or fill[fidx[0]][0] <= giter[0]
                ):
                    fill[fidx[0]][1]()
                    fidx[0] += 1
                if fidx[0] >= len(fill):
                    done[0] = True

            giter = [0]
            pend = []  # deferred PE/DVE work, drained ~2 iterations later

            def tick():
                giter[0] += 1
                pull_due()
                while len(pend) > 2:
                    pend.pop(0)()

            # ---------- attention units ----------
            def unit(b, hh, qh):
                nq = QCH[b][qh]
                if nq == 0:
                    return
                lo = hh * 64
                evt = evp.tile([65, 1024], f32, tag="ev", name="evps")
                ktn = KTN[b]

                def ev_mm(ktile, et_t):
                    def emit():
                        vsl = vpos[b][:, ktile, lo:lo + 65]
                        for cc in range(nq):
                            nc.tensor.matmul(
                                evt[:, cc * 512:(cc + 1) * 512],
                                vsl,
                                et_t[:, cc * 512:(cc + 1) * 512],
                                start=(ktile == 0),
                                stop=(ktile == ktn - 1),
                            )
                    return emit

                def writeout():
                    def emit():
                        stage = evs.tile([65, 1024], f32, name="evstage")
                        nc.vector.tensor_copy(stage[:], evt[:])
                        nc.gpsimd.dma_start(out=ev_d[b, hh, qh], in_=stage[:])
                    return emit

                for ktile in range(ktn):
                    ring_t = ring.tile([128, 1024], f32, tag="ring", name="sc_ps")
                    for cc in range(nq):
                        nc.tensor.matmul(
                            ring_t[:, cc * 512:(cc + 1) * 512],
                            kt[b][lo:lo + 64, ktile * 128:(ktile + 1) * 128],
                            qt[b][lo:lo + 64,
                                  qh * 1024 + cc * 512:qh * 1024 + (cc + 1) * 512],
                            start=True,
                            stop=True,
                        )
                    et_t = etp.tile([128, 1024], f16, name="et")
                    nc.scalar.activation(
                        out=et_t[:, 0:nq * 512],
                        in_=ring_t[:, 0:nq * 512],
                        func=mybir.ActivationFunctionType.Exp,
                        scale=0.125,
                    )
                    pend.append(ev_mm(ktile, et_t))
                    tick()
                pend.append(writeout())

            for b in range(B):
                for hh in range(2):
                    for qh in range(2):
                        unit(b, hh, qh)
            for p in pend:
                p()
            if not done[0]:
                pull_due(force=True)

    nc.compile()
    return nc


def _make_in_maps(x, L, Wq, Wk, Wv):
    x = np.asarray(x, dtype=np.float32)
    L = np.asarray(L)
    ident = np.eye(128, dtype=np.float16)
    xt = np.empty((B, 2, 8, 128, 1024), dtype=np.float16)
    for b in range(B):
        smask = (np.arange(S) < int(L[b])).astype(np.float32)
        xb = (x[b].T * smask[None, :]).reshape(8, 128, 2, 1024).astype(np.float16)
        xt[b] = xb.transpose(2, 0, 1, 3)
    in_maps = []
    for core in range(NCORES):
        m = {"xt": xt, "ident": ident}
        for nm, W in (("wq", Wq), ("wk", Wk), ("wv", Wv)):
            ws = np.asarray(W, dtype=np.float32)[core * 128:(core + 1) * 128, :].T
            m[nm] = np.ascontiguousarray(ws.reshape(8, 128, 128), dtype=np.float16)
        in_maps.append(m)
    return in_maps


def _postprocess(results, L, bv, Wo, bo):
    L = np.asarray(L)
    KTN, QCH, _, _ = _bounds(L)
    pooled = np.zeros((B, HDIM), dtype=np.float32)
    for core in range(NCORES):
        ev = np.asarray(results[core]["ev"])  # [B, 2, 2, 65, 1024]
        for b in range(B):
            Lb = int(L[b])
            for hh in range(2):
                cols = []
                for qh in range(2):
                    nq = QCH[b][qh]
                    if nq:
                        cols.append(ev[b, hh, qh][:, :nq * 512])
                flat = np.concatenate(cols, axis=1)
                ncols = flat.shape[1]
                if hh == 0:
                    dims, den = flat[0:64], flat[64]
                else:
                    den, dims = flat[0], flat[1:65]
                den_true = den - np.float32(KTN[b] * 128 - Lb)
                valid = np.arange(ncols) < Lb
                r = np.where(valid, 1.0 / (Lb * den_true), 0.0).astype(np.float32)
                g = core * 2 + hh
                pooled[b, g * 64:(g + 1) * 64] = dims @ r
    pooled = pooled + np.asarray(bv, dtype=np.float32)[None, :]
    out = pooled @ np.asarray(Wo, dtype=np.float32).T + np.asarray(bo, np.float32)
    return out.astype(np.float32)


_RUN_KWARGS = {}


def kernel(x, L, Wq, Wk, Wv, bv, Wo, bo):
    from concourse.bass_utils import run_bass_kernel_spmd

    nc = _build_program(np.asarray(L))
    in_maps = _make_in_maps(x, L, Wq, Wk, Wv)
    res = run_bass_kernel_spmd(nc, in_maps, list(range(NCORES)), **_RUN_KWARGS)
    kernel.last_results = res
    return _postprocess(res.results, L, bv, Wo, bo)


# revision 10
# speedup vs baseline: 1.5907x; 1.0123x over previous
"""Trainium2 Bass kernel v5 for AttentionAggregator (B=4, S=2048, H=1024, 16 heads).

Sharding (L-balanced): core c handles heads (2c, 2c+1) of EVERY batch; all
cores run the identical program, per-batch loop bounds specialized to L.

Transposed attention per (batch, head, query-half) unit:
  scoresT[kpos, q] = kT_tile.T @ q ; ET = exp(scoresT/8) -> f16 SBUF
  EV[d|den, q] += [vpos | ones].T @ ET   (PSUM, accumulated over k-tiles)
Ones-column gives softmax denominators. Host does normalization/pool/Wo.

v5 pipeline refinements over v4:
  - EV matmuls trail the score matmuls by TWO k-tiles so they never wait on
    the exp on the in-order PE queue.
  - Projections/transposes are interleaved as fine-grained per-t steps (2 MMs
    each) instead of 16-MM bursts, keeping the ACT stream fed.
  - Input DMAs spread across engine queues; lead-in starts after only the
    first half of batch 0.
"""

import numpy as np

S = 2048
HDIM = 1024
B = 4
NCORES = 8


def _bounds(L):
    ktn = [max(1, -(-int(l) // 128)) for l in L]
    qch = []
    for l in L:
        l = int(l)
        qch.append([
            max(0, min(2, -(-min(l, 1024) // 512))),
            max(0, min(2, -(-(l - 1024) // 512))) if l > 1024 else 0,
        ])
    pch = [max(1, -(-(k * 128) // 512)) for k in ktn]
    qpch = [q[0] + q[1] for q in qch]
    return ktn, qch, pch, qpch


def _build_program(L, debug=False):
    import concourse.mybir as mybir
    from concourse import bacc, tile

    f32 = mybir.dt.float32
    f16 = mybir.dt.float16
    nc = bacc.Bacc("TRN2", target_bir_lowering=False, debug=debug)

    KTN, QCH, PCH, QPCH = _bounds(L)

    xt_d = nc.dram_tensor("xt", [B, 2, 128, 8, 1024], f16, kind="ExternalInput")
    wq_d = nc.dram_tensor("wq", [128, 8, 128], f16, kind="ExternalInput")
    wk_d = nc.dram_tensor("wk", [128, 8, 128], f16, kind="ExternalInput")
    wv_d = nc.dram_tensor("wv", [128, 8, 128], f16, kind="ExternalInput")
    id_d = nc.dram_tensor("ident", [128, 128], f16, kind="ExternalInput")
    ev_d = nc.dram_tensor("ev", [B, 2, 2, 65, 1024], f32, kind="ExternalOutput")

    with tile.TileContext(nc) as tc:
        with (
            tc.tile_pool(name="const", bufs=1) as const,
            tc.tile_pool(name="xp", bufs=2) as xp,
            tc.tile_pool(name="qk", bufs=1) as qk,
            tc.tile_pool(name="vtp", bufs=2) as vtp,
            tc.tile_pool(name="etp", bufs=6) as etp,
            tc.tile_pool(name="evs", bufs=2) as evs,
            tc.tile_pool(name="ring", bufs=2, space="PSUM") as ring,
            tc.tile_pool(name="evp", bufs=1, space="PSUM") as evp,
            tc.tile_pool(name="pjp", bufs=1, space="PSUM") as pjp,
        ):
            wsb = {
                nm: const.tile([128, 8, 128], f16, name=f"{nm}_sb")
                for nm in ("wq", "wk", "wv")
            }
            idsb = const.tile([128, 128], f16, name="idsb")

            xsb = {}

            def dma_x(b, halves=(0, 1)):
                if b not in xsb:
                    xsb[b] = xp.tile([128, 8, S], f16, name="xtile")
                nch = max(PCH[b], QPCH[b])
                engs = [nc.sync, nc.gpsimd, nc.scalar] if b == 0 else [nc.sync]
                shards = [(0, 3), (3, 6), (6, 8)] if b == 0 else [(0, 8)]
                for half in halves:
                    lo, hi = half * 1024, min(nch * 512, (half + 1) * 1024)
                    if hi <= lo:
                        continue
                    for qi, (tl, th) in enumerate(shards):
                        engs[qi % len(engs)].dma_start(
                            out=xsb[b][:, tl:th, lo:hi],
                            in_=xt_d[b, half, :, tl:th, 0:hi - lo],
                        )

            # batch-0 first half + ident first so the lead-in starts fast
            nc.scalar.dma_start(out=idsb[:], in_=id_d[:])
            dma_x(0, halves=(0,))
            # warm the PE clock gate throughout the x DMA window: each burst
            # waits for one x shard to land, keeping the activity monitor busy
            wu = pjp.tile([128, 1024], f32, tag="pj", name="warmup")
            for t in range(8):
                for i in range(8):
                    nc.tensor.matmul(
                        wu[:, 0:128], idsb[:], xsb[0][:, t, 0:128],
                        start=True, stop=True,
                    )
            nc.scalar.dma_start(out=wsb["wq"][:], in_=wq_d[:])
            nc.scalar.dma_start(out=wsb["wk"][:], in_=wk_d[:])
            nc.scalar.dma_start(out=wsb["wv"][:], in_=wv_d[:])
            dma_x(0, halves=(1,))
            dma_x(1)

            qt = [qk.tile([128, S], f16, name=f"qt{b}") for b in range(B)]
            kt = [qk.tile([128, S], f16, name=f"kt{b}") for b in range(B)]
            vpos = [qk.tile([128, 16, 129], f16, name=f"vpos{b}") for b in range(B)]
            vt_cur = {}

            # ---------- fine-grained projection / transpose steps ----------
            def proj_steps(nm, b, half, dst_fn):
                """Yield per-t matmul steps + a final copy step for one
                1024-column half of a projection."""
                nch = PCH[b] if nm != "wq" else QPCH[b]
                ccs = [c for c in (0, 1) if half * 2 + c < nch]
                if not ccs:
                    return
                cell = {}

                def step_t(t):
                    def emit():
                        if t == 0:
                            cell["ps"] = pjp.tile(
                                [128, 1024], f32, tag="pj", name="proj_ps"
                            )
                        ps = cell["ps"]
                        for cc in ccs:
                            o = half * 1024 + cc * 512
                            nc.tensor.matmul(
                                ps[:, cc * 512:(cc + 1) * 512],
                                wsb[nm][:, t],
                                xsb[b][:, t, o:o + 512],
                                start=(t == 0),
                                stop=(t == 7),
                            )
                    return emit

                for t in range(8):
                    yield step_t(t)

                def copy_step():
                    ps = cell["ps"]
                    dst = dst_fn()
                    for cc in ccs:
                        o = half * 1024 + cc * 512
                        nc.vector.tensor_copy(
                            dst[:, o:o + 512], ps[:, cc * 512:(cc + 1) * 512]
                        )
                yield copy_step

            def v_alloc(b):
                def emit():
                    vt_cur[b] = vtp.tile([128, S], f16, name="vt")
                return emit

            def transpose_steps(b, grp):
                kts = list(range(grp * 8, min((grp + 1) * 8, KTN[b])))
                if not kts:
                    return
                cell = {}

                def tstep(sub):
                    def emit():
                        if sub == 0:
                            cell["tps"] = pjp.tile(
                                [128, 8, 128], f16, tag="pj", name="tps"
                            )
                        tps = cell["tps"]
                        for i in range(sub * 4, min((sub + 1) * 4, len(kts))):
                            nc.tensor.transpose(
                                tps[:, i],
                                vt_cur[b][:, kts[i] * 128:(kts[i] + 1) * 128],
                                idsb[:],
                            )
                    return emit

                yield tstep(0)
                if len(kts) > 4:
                    yield tstep(1)

                def copy_step():
                    tps = cell["tps"]
                    n = len(kts)
                    dst = vpos[b]
                    nc.vector.tensor_copy(
                        dst[:, kts[0]:kts[0] + n, 0:64], tps[:, 0:n, 0:64]
                    )
                    nc.vector.tensor_copy(
                        dst[:, kts[0]:kts[0] + n, 65:129], tps[:, 0:n, 64:128]
                    )
                    nc.vector.memset(dst[:, kts[0]:kts[0] + n, 64:65], 1.0)
                yield copy_step

            def batch_steps(b, first):
                """first: the part needed before attention on b can start
                (half 0 of q/k/v + transpose grp 0); rest comes via fill."""
                steps = []
                if first:
                    steps.extend(proj_steps("wq", b, 0, lambda b=b: qt[b]))
                    steps.extend(proj_steps("wk", b, 0, lambda b=b: kt[b]))
                    steps.append(v_alloc(b))
                    steps.extend(proj_steps("wv", b, 0, lambda b=b: vt_cur[b]))
                    steps.extend(transpose_steps(b, 0))
                else:
                    steps.extend(proj_steps("wk", b, 1, lambda b=b: kt[b]))
                    steps.extend(proj_steps("wv", b, 1, lambda b=b: vt_cur[b]))
                    steps.extend(transpose_steps(b, 1))
                    steps.extend(proj_steps("wq", b, 1, lambda b=b: qt[b]))
                return steps

            # lead-in: only q/k first halves of batch 0 emitted contiguously;
            # v+transposes land in the first two iterations via the fill
            lead = []
            lead.extend(proj_steps("wq", 0, 0, lambda: qt[0]))
            lead.extend(proj_steps("wk", 0, 0, lambda: kt[0]))
            for s in lead:
                s()

            def due_range(steps, lo, hi):
                n = max(1, len(steps))
                return [
                    (lo + (hi - lo) * i // n, s) for i, s in enumerate(steps)
                ]

            fill = []
            vpart = [v_alloc(0)]
            vpart.extend(proj_steps("wv", 0, 0, lambda: vt_cur[0]))
            vpart.extend(transpose_steps(0, 0))
            fill += due_range(vpart, 0, 1)           # needed by EV(kt0) ~iter 2
            rest0 = batch_steps(0, False)
            fill += due_range(rest0, 2, 14)          # k/v h1 by kt 8, q h1 by 16
            # per-batch unit-start iterations
            ustart = [0]
            for b in range(B):
                ustart.append(
                    ustart[-1]
                    + KTN[b] * sum(2 for q in range(2) if QCH[b][q] > 0)
                )
            for b in range(1, B):
                steps = batch_steps(b, True) + batch_steps(b, False)
                lo = ustart[b - 1] + (18 if b == 1 else 2)
                fill += due_range(steps, lo, ustart[b] - 10)
            dma_x(2)
            dma_x(3)
            fill.sort(key=lambda p: p[0])
            fidx = [0]
            done = [False]

            def pull_due(force=False):
                while fidx[0] < len(fill) and (
                    force or fill[fidx[0]][0] <= giter[0]
                ):
                    fill[fidx[0]][1]()
                    fidx[0] += 1
                if fidx[0] >= len(fill):
                    done[0] = True

            giter = [0]
            pend = []  # deferred PE/DVE work, drained ~2 iterations later

            def tick():
                giter[0] += 1
                pull_due()
                while len(pend) > 2:
                    pend.pop(0)()

            # ---------- attention units ----------
            def unit(b, hh, qh):
                nq = QCH[b][qh]
                if nq == 0:
                    return
                lo = hh * 64
                evt = evp.tile([65, 1024], f32, tag="ev", name="evps")
                ktn = KTN[b]

                def ev_mm(ktile, et_t):
                    def emit():
                        vsl = vpos[b][:, ktile, lo:lo + 65]
                        for cc in range(nq):
                            nc.tensor.matmul(
                                evt[:, cc * 512:(cc + 1) * 512],
                                vsl,
                                et_t[:, cc * 512:(cc + 1) * 512],
                                start=(ktile == 0),
                                stop=(ktile == ktn - 1),
                            )
                    return emit

                def writeout():
                    def emit():
                        stage = evs.tile([65, 1024], f32, name="evstage")
                        nc.vector.tensor_copy(stage[:], evt[:])
                        nc.gpsimd.dma_start(out=ev_d[b, hh, qh], in_=stage[:])
                    return emit

                for ktile in range(ktn):
                    ring_t = ring.tile([128, 1024], f32, tag="ring", name="sc_ps")
                    for cc in range(nq):
                        nc.tensor.matmul(
                            ring_t[:, cc * 512:(cc + 1) * 512],
                            kt[b][lo:lo + 64, ktile * 128:(ktile + 1) * 128],
                            qt[b][lo:lo + 64,
                                  qh * 1024 + cc * 512:qh * 1024 + (cc + 1) * 512],
                            start=True,
                            stop=True,
                        )
                    et_t = etp.tile([128, 1024], f16, name="et")
                    nc.scalar.activation(
                        out=et_t[:, 0:nq * 512],
                        in_=ring_t[:, 0:nq * 512],
                        func=mybir.ActivationFunctionType.Exp,
                        scale=0.125,
                    )
                    pend.append(ev_mm(ktile, et_t))
                    tick()
                pend.append(writeout())

            for b in range(B):
                for hh in range(2):
                    for qh in range(2):
                        unit(b, hh, qh)
            for p in pend:
                p()
            if not done[0]:
                pull_due(force=True)

    nc.compile()
    return nc


def _make_in_maps(x, L, Wq, Wk, Wv):
    x = np.asarray(x, dtype=np.float32)
    L = np.asarray(L)
    ident = np.eye(128, dtype=np.float16)
    xt = np.empty((B, 2, 128, 8, 1024), dtype=np.float16)
    for b in range(B):
        smask = (np.arange(S) < int(L[b])).astype(np.float32)
        xb = (x[b].T * smask[None, :]).reshape(8, 128, 2, 1024).astype(np.float16)
        xt[b] = xb.transpose(2, 1, 0, 3)
    in_maps = []
    for core in range(NCORES):
        m = {"xt": xt, "ident": ident}
        for nm, W in (("wq", Wq), ("wk", Wk), ("wv", Wv)):
            ws = np.asarray(W, dtype=np.float32)[core * 128:(core + 1) * 128, :].T
            m[nm] = np.ascontiguousarray(
                ws.reshape(8, 128, 128).transpose(1, 0, 2), dtype=np.float16
            )
        in_maps.append(m)
    return in_maps


def _postprocess(results, L, bv, Wo, bo):
    L = np.asarray(L)
    KTN, QCH, _, _ = _bounds(L)
    pooled = np.zeros((B, HDIM), dtype=np.float32)
    for core in range(NCORES):
        ev = np.asarray(results[core]["ev"])  # [B, 2, 2, 65, 1024]
        for b in range(B):
            Lb = int(L[b])
            for hh in range(2):
                cols = []
                for qh in range(2):
                    nq = QCH[b][qh]
                    if nq:
                        cols.append(ev[b, hh, qh][:, :nq * 512])
                flat = np.concatenate(cols, axis=1)
                ncols = flat.shape[1]
                if hh == 0:
                    dims, den = flat[0:64], flat[64]
                else:
                    den, dims = flat[0], flat[1:65]
                den_true = den - np.float32(KTN[b] * 128 - Lb)
                valid = np.arange(ncols) < Lb
                r = np.where(valid, 1.0 / (Lb * den_true), 0.0).astype(np.float32)
                g = core * 2 + hh
                pooled[b, g * 64:(g + 1) * 64] = dims @ r
    pooled = pooled + np.asarray(bv, dtype=np.float32)[None, :]
    out = pooled @ np.asarray(Wo, dtype=np.float32).T + np.asarray(bo, np.float32)
    return out.astype(np.float32)


_RUN_KWARGS = {}


def kernel(x, L, Wq, Wk, Wv, bv, Wo, bo):
    from concourse.bass_utils import run_bass_kernel_spmd

    nc = _build_program(np.asarray(L))
    in_maps = _make_in_maps(x, L, Wq, Wk, Wv)
    res = run_bass_kernel_spmd(nc, in_maps, list(range(NCORES)), **_RUN_KWARGS)
    kernel.last_results = res
    return _postprocess(res.results, L, bv, Wo, bo)


# revision 11
# speedup vs baseline: 1.7618x; 1.1076x over previous
"""Trainium2 Bass kernel v5 for AttentionAggregator (B=4, S=2048, H=1024, 16 heads).

Sharding (L-balanced): core c handles heads (2c, 2c+1) of EVERY batch; all
cores run the identical program, per-batch loop bounds specialized to L.

Transposed attention per (batch, head, query-half) unit:
  scoresT[kpos, q] = kT_tile.T @ q ; ET = exp(scoresT/8) -> f16 SBUF
  EV[d|den, q] += [vpos | ones].T @ ET   (PSUM, accumulated over k-tiles)
Ones-column gives softmax denominators. Host does normalization/pool/Wo.

v5 pipeline refinements over v4:
  - EV matmuls trail the score matmuls by TWO k-tiles so they never wait on
    the exp on the in-order PE queue.
  - Projections/transposes are interleaved as fine-grained per-t steps (2 MMs
    each) instead of 16-MM bursts, keeping the ACT stream fed.
  - Input DMAs spread across engine queues; lead-in starts after only the
    first half of batch 0.
"""

import numpy as np

S = 2048
HDIM = 1024
B = 4
NCORES = 8


def _bounds(L):
    ktn = [max(1, -(-int(l) // 128)) for l in L]
    qch = []
    for l in L:
        l = int(l)
        qch.append([
            max(0, min(2, -(-min(l, 1024) // 512))),
            max(0, min(2, -(-(l - 1024) // 512))) if l > 1024 else 0,
        ])
    pch = [max(1, -(-(k * 128) // 512)) for k in ktn]
    qpch = [q[0] + q[1] for q in qch]
    return ktn, qch, pch, qpch


def _build_program(L, debug=False):
    import concourse.mybir as mybir
    from concourse import bacc, tile

    f32 = mybir.dt.float32
    f16 = mybir.dt.float16
    nc = bacc.Bacc("TRN2", target_bir_lowering=False, debug=debug)

    KTN, QCH, PCH, QPCH = _bounds(L)

    xt_d = nc.dram_tensor("xt", [B, 2, 128, 8, 1024], f16, kind="ExternalInput")
    wq_d = nc.dram_tensor("wq", [128, 8, 128], f16, kind="ExternalInput")
    wk_d = nc.dram_tensor("wk", [128, 8, 128], f16, kind="ExternalInput")
    wv_d = nc.dram_tensor("wv", [128, 8, 128], f16, kind="ExternalInput")
    id_d = nc.dram_tensor("ident", [128, 128], f16, kind="ExternalInput")
    ev_d = nc.dram_tensor("ev", [B, 2, 2, 65, 1024], f32, kind="ExternalOutput")

    with tile.TileContext(nc) as tc:
        with (
            tc.tile_pool(name="const", bufs=1) as const,
            tc.tile_pool(name="xp", bufs=2) as xp,
            tc.tile_pool(name="qk", bufs=1) as qk,
            tc.tile_pool(name="vtp", bufs=2) as vtp,
            tc.tile_pool(name="etp", bufs=6) as etp,
            tc.tile_pool(name="evs", bufs=2) as evs,
            tc.tile_pool(name="ring", bufs=2, space="PSUM") as ring,
            tc.tile_pool(name="evp", bufs=1, space="PSUM") as evp,
            tc.tile_pool(name="pjp", bufs=1, space="PSUM") as pjp,
        ):
            wsb = {
                nm: const.tile([128, 8, 128], f16, name=f"{nm}_sb")
                for nm in ("wq", "wk", "wv")
            }
            idsb = const.tile([128, 128], f16, name="idsb")

            xsb = {}

            def dma_x(b, halves=(0, 1)):
                if b not in xsb:
                    xsb[b] = xp.tile([128, 8, S], f16, name="xtile")
                nch = max(PCH[b], QPCH[b])
                engs = [nc.sync, nc.gpsimd, nc.scalar] if b == 0 else [nc.sync]
                shards = [(0, 3), (3, 6), (6, 8)] if b == 0 else [(0, 8)]
                for half in halves:
                    lo, hi = half * 1024, min(nch * 512, (half + 1) * 1024)
                    if hi <= lo:
                        continue
                    for qi, (tl, th) in enumerate(shards):
                        engs[qi % len(engs)].dma_start(
                            out=xsb[b][:, tl:th, lo:hi],
                            in_=xt_d[b, half, :, tl:th, 0:hi - lo],
                        )

            # batch-0 first half + ident first so the lead-in starts fast
            nc.scalar.dma_start(out=idsb[:], in_=id_d[:])
            dma_x(0, halves=(0,))
            # warm the PE clock gate throughout the x DMA window: each burst
            # waits for one x shard to land, keeping the activity monitor busy
            wu = pjp.tile([128, 1024], f32, tag="pj", name="warmup")
            for t in range(8):
                for i in range(8):
                    nc.tensor.matmul(
                        wu[:, 0:128], idsb[:], xsb[0][:, t, 0:128],
                        start=True, stop=True,
                    )
            nc.scalar.dma_start(out=wsb["wq"][:], in_=wq_d[:])
            nc.scalar.dma_start(out=wsb["wk"][:], in_=wk_d[:])
            nc.scalar.dma_start(out=wsb["wv"][:], in_=wv_d[:])
            dma_x(0, halves=(1,))
            dma_x(1)

            qt = [qk.tile([128, S], f16, name=f"qt{b}") for b in range(B)]
            ktE = [qk.tile([128, S], f16, name=f"ktE{b}") for b in range(B)]
            ktO = [qk.tile([128, S], f16, name=f"ktO{b}") for b in range(B)]
            vpos = [qk.tile([128, 16, 129], f16, name=f"vpos{b}") for b in range(B)]
            for b in range(B):
                nc.vector.memset(ktE[b][64:128, :], 0.0)
                nc.vector.memset(ktO[b][0:64, :], 0.0)
            vt_cur = {}

            # ---------- fine-grained projection / transpose steps ----------
            def proj_steps(nm, b, half, dst_fn):
                """Yield per-t matmul steps + a final copy step for one
                1024-column half of a projection."""
                nch = PCH[b] if nm != "wq" else QPCH[b]
                ccs = [c for c in (0, 1) if half * 2 + c < nch]
                if not ccs:
                    return
                cell = {}

                def step_t(t):
                    def emit():
                        if t == 0:
                            cell["ps"] = pjp.tile(
                                [128, 1024], f32, tag="pj", name="proj_ps"
                            )
                        ps = cell["ps"]
                        for cc in ccs:
                            o = half * 1024 + cc * 512
                            nc.tensor.matmul(
                                ps[:, cc * 512:(cc + 1) * 512],
                                wsb[nm][:, t],
                                xsb[b][:, t, o:o + 512],
                                start=(t == 0),
                                stop=(t == 7),
                            )
                    return emit

                for t in range(8):
                    yield step_t(t)

                def copy_step():
                    ps = cell["ps"]
                    dst = dst_fn()
                    for cc in ccs:
                        o = half * 1024 + cc * 512
                        if nm == "wk":
                            nc.vector.tensor_copy(
                                dst[0][0:64, o:o + 512],
                                ps[0:64, cc * 512:(cc + 1) * 512],
                            )
                            nc.vector.tensor_copy(
                                dst[1][64:128, o:o + 512],
                                ps[64:128, cc * 512:(cc + 1) * 512],
                            )
                        else:
                            nc.vector.tensor_copy(
                                dst[:, o:o + 512], ps[:, cc * 512:(cc + 1) * 512]
                            )
                yield copy_step

            def v_alloc(b):
                def emit():
                    vt_cur[b] = vtp.tile([128, S], f16, name="vt")
                return emit

            def transpose_steps(b, grp):
                kts = list(range(grp * 8, min((grp + 1) * 8, KTN[b])))
                if not kts:
                    return
                cell = {}

                def tstep(sub):
                    def emit():
                        if sub == 0:
                            cell["tps"] = pjp.tile(
                                [128, 8, 128], f16, tag="pj", name="tps"
                            )
                        tps = cell["tps"]
                        for i in range(sub * 4, min((sub + 1) * 4, len(kts))):
                            nc.tensor.transpose(
                                tps[:, i],
                                vt_cur[b][:, kts[i] * 128:(kts[i] + 1) * 128],
                                idsb[:],
                            )
                    return emit

                yield tstep(0)
                if len(kts) > 4:
                    yield tstep(1)

                def copy_step():
                    tps = cell["tps"]
                    n = len(kts)
                    dst = vpos[b]
                    nc.vector.tensor_copy(
                        dst[:, kts[0]:kts[0] + n, 0:64], tps[:, 0:n, 0:64]
                    )
                    nc.vector.tensor_copy(
                        dst[:, kts[0]:kts[0] + n, 65:129], tps[:, 0:n, 64:128]
                    )
                    nc.vector.memset(dst[:, kts[0]:kts[0] + n, 64:65], 1.0)
                yield copy_step

            def batch_steps(b, first):
                """first: the part needed before attention on b can start
                (half 0 of q/k/v + transpose grp 0); rest comes via fill."""
                steps = []
                if first:
                    steps.extend(proj_steps("wq", b, 0, lambda b=b: qt[b]))
                    steps.extend(proj_steps("wk", b, 0, lambda b=b: (ktE[b], ktO[b])))
                    steps.append(v_alloc(b))
                    steps.extend(proj_steps("wv", b, 0, lambda b=b: vt_cur[b]))
                    steps.extend(transpose_steps(b, 0))
                else:
                    steps.extend(proj_steps("wk", b, 1, lambda b=b: (ktE[b], ktO[b])))
                    steps.extend(proj_steps("wv", b, 1, lambda b=b: vt_cur[b]))
                    steps.extend(transpose_steps(b, 1))
                    steps.extend(proj_steps("wq", b, 1, lambda b=b: qt[b]))
                return steps

            # lead-in: only q/k first halves of batch 0 emitted contiguously;
            # v+transposes land in the first two iterations via the fill
            lead = []
            lead.extend(proj_steps("wq", 0, 0, lambda: qt[0]))
            lead.extend(proj_steps("wk", 0, 0, lambda: (ktE[0], ktO[0])))
            for s in lead:
                s()

            def due_range(steps, lo, hi):
                n = max(1, len(steps))
                return [
                    (lo + (hi - lo) * i // n, s) for i, s in enumerate(steps)
                ]

            fill = []
            vpart = [v_alloc(0)]
            vpart.extend(proj_steps("wv", 0, 0, lambda: vt_cur[0]))
            vpart.extend(transpose_steps(0, 0))
            fill += due_range(vpart, 0, 1)           # needed by EV(kt0) ~iter 2
            rest0 = batch_steps(0, False)
            fill += due_range(rest0, 2, 14)          # k/v h1 by kt 8, q h1 by 16
            # per-batch unit-start iterations
            ustart = [0]
            for b in range(B):
                ustart.append(
                    ustart[-1]
                    + KTN[b] * sum(2 for q in range(2) if QCH[b][q] > 0)
                )
            for b in range(1, B):
                steps = batch_steps(b, True) + batch_steps(b, False)
                lo = ustart[b - 1] + (18 if b == 1 else 2)
                fill += due_range(steps, lo, ustart[b] - 10)
            dma_x(2)
            dma_x(3)
            fill.sort(key=lambda p: p[0])
            fidx = [0]
            done = [False]

            def pull_due(force=False):
                while fidx[0] < len(fill) and (
                    force or fill[fidx[0]][0] <= giter[0]
                ):
                    fill[fidx[0]][1]()
                    fidx[0] += 1
                if fidx[0] >= len(fill):
                    done[0] = True

            giter = [0]
            pend = []  # deferred PE/DVE work, drained ~2 iterations later

            def tick():
                giter[0] += 1
                pull_due()
                while len(pend) > 2:
                    pend.pop(0)()

            # ---------- attention units ----------
            def unit(b, hh, qh):
                nq = QCH[b][qh]
                if nq == 0:
                    return
                lo = hh * 64
                evt = evp.tile([65, 1024], f32, tag="ev", name="evps")
                ktn = KTN[b]

                def ev_mm(ktile, et_t):
                    def emit():
                        vsl = vpos[b][:, ktile, lo:lo + 65]
                        for cc in range(nq):
                            nc.tensor.matmul(
                                evt[:, cc * 512:(cc + 1) * 512],
                                vsl,
                                et_t[:, cc * 512:(cc + 1) * 512],
                                start=(ktile == 0),
                                stop=(ktile == ktn - 1),
                            )
                    return emit

                def writeout():
                    def emit():
                        stage = evs.tile([65, 1024], f32, name="evstage")
                        nc.vector.tensor_copy(stage[:], evt[:])
                        nc.gpsimd.dma_start(out=ev_d[b, hh, qh], in_=stage[:])
                    return emit

                for ktile in range(ktn):
                    ring_t = ring.tile([128, 1024], f32, tag="ring", name="sc_ps")
                    ktz = ktE[b] if hh == 0 else ktO[b]
                    for cc in range(nq):
                        nc.tensor.matmul(
                            ring_t[:, cc * 512:(cc + 1) * 512],
                            ktz[:, ktile * 128:(ktile + 1) * 128],
                            qt[b][:,
                                  qh * 1024 + cc * 512:qh * 1024 + (cc + 1) * 512],
                            start=True,
                            stop=True,
                        )
                    et_t = etp.tile([128, 1024], f16, name="et")
                    nc.scalar.activation(
                        out=et_t[:, 0:nq * 512],
                        in_=ring_t[:, 0:nq * 512],
                        func=mybir.ActivationFunctionType.Exp,
                        scale=0.125,
                    )
                    pend.append(ev_mm(ktile, et_t))
                    tick()
                pend.append(writeout())

            for b in range(B):
                for hh in range(2):
                    for qh in range(2):
                        unit(b, hh, qh)
            for p in pend:
                p()
            if not done[0]:
                pull_due(force=True)

    nc.compile()
    return nc


def _make_in_maps(x, L, Wq, Wk, Wv):
    x = np.asarray(x, dtype=np.float32)
    L = np.asarray(L)
    ident = np.eye(128, dtype=np.float16)
    xt = np.empty((B, 2, 128, 8, 1024), dtype=np.float16)
    for b in range(B):
        smask = (np.arange(S) < int(L[b])).astype(np.float32)
        xb = (x[b].T * smask[None, :]).reshape(8, 128, 2, 1024).astype(np.float16)
        xt[b] = xb.transpose(2, 1, 0, 3)
    in_maps = []
    for core in range(NCORES):
        m = {"xt": xt, "ident": ident}
        for nm, W in (("wq", Wq), ("wk", Wk), ("wv", Wv)):
            ws = np.asarray(W, dtype=np.float32)[core * 128:(core + 1) * 128, :].T
            m[nm] = np.ascontiguousarray(
                ws.reshape(8, 128, 128).transpose(1, 0, 2), dtype=np.float16
            )
        in_maps.append(m)
    return in_maps


def _postprocess(results, L, bv, Wo, bo):
    L = np.asarray(L)
    KTN, QCH, _, _ = _bounds(L)
    pooled = np.zeros((B, HDIM), dtype=np.float32)
    for core in range(NCORES):
        ev = np.asarray(results[core]["ev"])  # [B, 2, 2, 65, 1024]
        for b in range(B):
            Lb = int(L[b])
            for hh in range(2):
                cols = []
                for qh in range(2):
                    nq = QCH[b][qh]
                    if nq:
                        cols.append(ev[b, hh, qh][:, :nq * 512])
                flat = np.concatenate(cols, axis=1)
                ncols = flat.shape[1]
                if hh == 0:
                    dims, den = flat[0:64], flat[64]
                else:
                    den, dims = flat[0], flat[1:65]
                den_true = den - np.float32(KTN[b] * 128 - Lb)
                valid = np.arange(ncols) < Lb
                r = np.where(valid, 1.0 / (Lb * den_true), 0.0).astype(np.float32)
                g = core * 2 + hh
                pooled[b, g * 64:(g + 1) * 64] = dims @ r
    pooled = pooled + np.asarray(bv, dtype=np.float32)[None, :]
    out = pooled @ np.asarray(Wo, dtype=np.float32).T + np.asarray(bo, np.float32)
    return out.astype(np.float32)


_RUN_KWARGS = {}


def kernel(x, L, Wq, Wk, Wv, bv, Wo, bo):
    from concourse.bass_utils import run_bass_kernel_spmd

    nc = _build_program(np.asarray(L))
    in_maps = _make_in_maps(x, L, Wq, Wk, Wv)
    res = run_bass_kernel_spmd(nc, in_maps, list(range(NCORES)), **_RUN_KWARGS)
    kernel.last_results = res
    return _postprocess(res.results, L, bv, Wo, bo)


# revision 12
# speedup vs baseline: 1.7894x; 1.0157x over previous
"""Trainium2 Bass kernel v5 for AttentionAggregator (B=4, S=2048, H=1024, 16 heads).

Sharding (L-balanced): core c handles heads (2c, 2c+1) of EVERY batch; all
cores run the identical program, per-batch loop bounds specialized to L.

Transposed attention per (batch, head, query-half) unit:
  scoresT[kpos, q] = kT_tile.T @ q ; ET = exp(scoresT/8) -> f16 SBUF
  EV[d|den, q] += [vpos | ones].T @ ET   (PSUM, accumulated over k-tiles)
Ones-column gives softmax denominators. Host does normalization/pool/Wo.

v5 pipeline refinements over v4:
  - EV matmuls trail the score matmuls by TWO k-tiles so they never wait on
    the exp on the in-order PE queue.
  - Projections/transposes are interleaved as fine-grained per-t steps (2 MMs
    each) instead of 16-MM bursts, keeping the ACT stream fed.
  - Input DMAs spread across engine queues; lead-in starts after only the
    first half of batch 0.
"""

import numpy as np

S = 2048
HDIM = 1024
B = 4
NCORES = 8


def _bounds(L):
    ktn = [max(1, -(-int(l) // 128)) for l in L]
    qch = []
    for l in L:
        l = int(l)
        qch.append([
            max(0, min(2, -(-min(l, 1024) // 512))),
            max(0, min(2, -(-(l - 1024) // 512))) if l > 1024 else 0,
        ])
    pch = [max(1, -(-(k * 128) // 512)) for k in ktn]
    qpch = [q[0] + q[1] for q in qch]
    return ktn, qch, pch, qpch


def _build_program(L, debug=False):
    import concourse.mybir as mybir
    from concourse import bacc, tile

    f32 = mybir.dt.float32
    f16 = mybir.dt.float16
    nc = bacc.Bacc("TRN2", target_bir_lowering=False, debug=debug)

    KTN, QCH, PCH, QPCH = _bounds(L)

    xt_d = nc.dram_tensor("xt", [B, 2, 128, 8, 1024], f16, kind="ExternalInput")
    wq_d = nc.dram_tensor("wq", [128, 8, 128], f16, kind="ExternalInput")
    wk_d = nc.dram_tensor("wk", [128, 8, 128], f16, kind="ExternalInput")
    wv_d = nc.dram_tensor("wv", [128, 8, 128], f16, kind="ExternalInput")
    id_d = nc.dram_tensor("ident", [128, 128], f16, kind="ExternalInput")
    ev_d = nc.dram_tensor("ev", [B, 2, 2, 65, 1024], f32, kind="ExternalOutput")

    with tile.TileContext(nc) as tc:
        with (
            tc.tile_pool(name="const", bufs=1) as const,
            tc.tile_pool(name="xp", bufs=2) as xp,
            tc.tile_pool(name="qk", bufs=1) as qk,
            tc.tile_pool(name="vtp", bufs=2) as vtp,
            tc.tile_pool(name="etp", bufs=6) as etp,
            tc.tile_pool(name="evs", bufs=2) as evs,
            tc.tile_pool(name="ring", bufs=2, space="PSUM") as ring,
            tc.tile_pool(name="evp", bufs=1, space="PSUM") as evp,
            tc.tile_pool(name="pjp", bufs=1, space="PSUM") as pjp,
        ):
            wsb = {
                nm: const.tile([128, 8, 128], f16, name=f"{nm}_sb")
                for nm in ("wq", "wk", "wv")
            }
            idsb = const.tile([128, 128], f16, name="idsb")

            xsb = {}

            def dma_x(b, halves=(0, 1)):
                if b not in xsb:
                    xsb[b] = xp.tile([128, 8, S], f16, name="xtile")
                nch = max(PCH[b], QPCH[b])
                engs = [nc.sync, nc.gpsimd, nc.scalar] if b == 0 else [nc.sync]
                shards = [(0, 3), (3, 6), (6, 8)] if b == 0 else [(0, 8)]
                for half in halves:
                    lo, hi = half * 1024, min(nch * 512, (half + 1) * 1024)
                    if hi <= lo:
                        continue
                    for qi, (tl, th) in enumerate(shards):
                        engs[qi % len(engs)].dma_start(
                            out=xsb[b][:, tl:th, lo:hi],
                            in_=xt_d[b, half, :, tl:th, 0:hi - lo],
                        )

            # batch-0 first half + ident first so the lead-in starts fast
            nc.scalar.dma_start(out=idsb[:], in_=id_d[:])
            dma_x(0, halves=(0,))
            # warm the PE clock gate throughout the x DMA window: each burst
            # waits for one x shard to land, keeping the activity monitor busy
            wu = pjp.tile([128, 1024], f32, tag="pj", name="warmup")
            for t in range(8):
                for i in range(8):
                    nc.tensor.matmul(
                        wu[:, 0:128], idsb[:], xsb[0][:, t, 0:128],
                        start=True, stop=True,
                    )
            nc.scalar.dma_start(out=wsb["wq"][:], in_=wq_d[:])
            nc.scalar.dma_start(out=wsb["wk"][:], in_=wk_d[:])
            nc.scalar.dma_start(out=wsb["wv"][:], in_=wv_d[:])
            dma_x(0, halves=(1,))
            dma_x(1)

            qt = [qk.tile([128, S], f16, name=f"qt{b}") for b in range(B)]
            ktE = [qk.tile([128, S], f16, name=f"ktE{b}") for b in range(B)]
            ktO = [qk.tile([128, S], f16, name=f"ktO{b}") for b in range(B)]
            vpos = [qk.tile([128, 16, 129], f16, name=f"vpos{b}") for b in range(B)]
            for b in range(B):
                nc.vector.memset(ktE[b][64:128, :], 0.0)
                nc.vector.memset(ktO[b][0:64, :], 0.0)
            vt_cur = {}

            # ---------- fine-grained projection / transpose steps ----------
            def proj_steps(nm, b, half, dst_fn):
                """Yield per-t matmul steps + a final copy step for one
                1024-column half of a projection."""
                nch = PCH[b] if nm != "wq" else QPCH[b]
                ccs = [c for c in (0, 1) if half * 2 + c < nch]
                if not ccs:
                    return
                cell = {}

                def step_t(t):
                    def emit():
                        if t == 0:
                            cell["ps"] = pjp.tile(
                                [128, 1024], f32, tag="pj", name="proj_ps"
                            )
                        ps = cell["ps"]
                        for cc in ccs:
                            o = half * 1024 + cc * 512
                            nc.tensor.matmul(
                                ps[:, cc * 512:(cc + 1) * 512],
                                wsb[nm][:, t],
                                xsb[b][:, t, o:o + 512],
                                start=(t == 0),
                                stop=(t == 7),
                            )
                    return emit

                for t in range(8):
                    yield step_t(t)

                def copy_step():
                    ps = cell["ps"]
                    dst = dst_fn()
                    for cc in ccs:
                        o = half * 1024 + cc * 512
                        if nm == "wk":
                            nc.vector.tensor_copy(
                                dst[0][0:64, o:o + 512],
                                ps[0:64, cc * 512:(cc + 1) * 512],
                            )
                            nc.vector.tensor_copy(
                                dst[1][64:128, o:o + 512],
                                ps[64:128, cc * 512:(cc + 1) * 512],
                            )
                        else:
                            nc.vector.tensor_copy(
                                dst[:, o:o + 512], ps[:, cc * 512:(cc + 1) * 512]
                            )
                yield copy_step

            def v_alloc(b):
                def emit():
                    vt_cur[b] = vtp.tile([128, S], f16, name="vt")
                return emit

            def transpose_steps(b, grp):
                kts = list(range(grp * 8, min((grp + 1) * 8, KTN[b])))
                if not kts:
                    return
                cell = {}

                def tstep(sub):
                    def emit():
                        if sub == 0:
                            cell["tps"] = pjp.tile(
                                [128, 8, 128], f16, tag="pj", name="tps"
                            )
                        tps = cell["tps"]
                        for i in range(sub * 4, min((sub + 1) * 4, len(kts))):
                            nc.tensor.transpose(
                                tps[:, i],
                                vt_cur[b][:, kts[i] * 128:(kts[i] + 1) * 128],
                                idsb[:],
                            )
                    return emit

                yield tstep(0)
                if len(kts) > 4:
                    yield tstep(1)

                def copy_step():
                    tps = cell["tps"]
                    n = len(kts)
                    dst = vpos[b]
                    nc.vector.tensor_copy(
                        dst[:, kts[0]:kts[0] + n, 0:64], tps[:, 0:n, 0:64]
                    )
                    nc.vector.tensor_copy(
                        dst[:, kts[0]:kts[0] + n, 65:129], tps[:, 0:n, 64:128]
                    )
                    nc.vector.memset(dst[:, kts[0]:kts[0] + n, 64:65], 1.0)
                yield copy_step

            def batch_steps(b, first):
                """first: the part needed before attention on b can start
                (half 0 of q/k/v + transpose grp 0); rest comes via fill."""
                steps = []
                if first:
                    steps.extend(proj_steps("wq", b, 0, lambda b=b: qt[b]))
                    steps.extend(proj_steps("wk", b, 0, lambda b=b: (ktE[b], ktO[b])))
                    steps.append(v_alloc(b))
                    steps.extend(proj_steps("wv", b, 0, lambda b=b: vt_cur[b]))
                    steps.extend(transpose_steps(b, 0))
                else:
                    steps.extend(proj_steps("wk", b, 1, lambda b=b: (ktE[b], ktO[b])))
                    steps.extend(proj_steps("wv", b, 1, lambda b=b: vt_cur[b]))
                    steps.extend(transpose_steps(b, 1))
                    steps.extend(proj_steps("wq", b, 1, lambda b=b: qt[b]))
                return steps

            # lead-in: only q/k first halves of batch 0 emitted contiguously;
            # v+transposes land in the first two iterations via the fill
            lead = []
            lead.extend(proj_steps("wq", 0, 0, lambda: qt[0]))
            lead.extend(proj_steps("wk", 0, 0, lambda: (ktE[0], ktO[0])))
            for s in lead:
                s()

            def due_range(steps, lo, hi):
                n = max(1, len(steps))
                return [
                    (lo + (hi - lo) * i // n, s) for i, s in enumerate(steps)
                ]

            fill = []
            vpart = [v_alloc(0)]
            vpart.extend(proj_steps("wv", 0, 0, lambda: vt_cur[0]))
            vpart.extend(transpose_steps(0, 0))
            fill += due_range(vpart, 0, 1)           # needed by EV(kt0) ~iter 2
            rest0 = batch_steps(0, False)
            fill += due_range(rest0, 2, 14)          # k/v h1 by kt 8, q h1 by 16
            # per-batch unit-start iterations
            ustart = [0]
            for b in range(B):
                ustart.append(
                    ustart[-1]
                    + KTN[b] * sum(2 for q in range(2) if QCH[b][q] > 0)
                )
            for b in range(1, B):
                lo = ustart[b - 1] + (18 if b == 1 else 2)
                fill += due_range(batch_steps(b, True), lo, ustart[b] - 12)
                fill += due_range(
                    batch_steps(b, False), ustart[b] - 8, ustart[b] + 10
                )
            dma_x(2)
            dma_x(3)
            fill.sort(key=lambda p: p[0])
            fidx = [0]
            done = [False]

            def pull_due(force=False):
                while fidx[0] < len(fill) and (
                    force or fill[fidx[0]][0] <= giter[0]
                ):
                    fill[fidx[0]][1]()
                    fidx[0] += 1
                if fidx[0] >= len(fill):
                    done[0] = True

            giter = [0]
            pend = []  # deferred PE/DVE work, drained ~2 iterations later

            def tick():
                giter[0] += 1
                pull_due()
                while len(pend) > 2:
                    pend.pop(0)()

            # ---------- attention units ----------
            def unit(b, hh, qh):
                nq = QCH[b][qh]
                if nq == 0:
                    return
                lo = hh * 64
                evt = evp.tile([65, 1024], f32, tag="ev", name="evps")
                ktn = KTN[b]

                def ev_mm(ktile, et_t):
                    def emit():
                        vsl = vpos[b][:, ktile, lo:lo + 65]
                        for cc in range(nq):
                            nc.tensor.matmul(
                                evt[:, cc * 512:(cc + 1) * 512],
                                vsl,
                                et_t[:, cc * 512:(cc + 1) * 512],
                                start=(ktile == 0),
                                stop=(ktile == ktn - 1),
                            )
                    return emit

                def writeout():
                    def emit():
                        stage = evs.tile([65, 1024], f32, name="evstage")
                        nc.vector.tensor_copy(stage[:], evt[:])
                        nc.gpsimd.dma_start(out=ev_d[b, hh, qh], in_=stage[:])
                    return emit

                for ktile in range(ktn):
                    ring_t = ring.tile([128, 1024], f32, tag="ring", name="sc_ps")
                    ktz = ktE[b] if hh == 0 else ktO[b]
                    for cc in range(nq):
                        nc.tensor.matmul(
                            ring_t[:, cc * 512:(cc + 1) * 512],
                            ktz[:, ktile * 128:(ktile + 1) * 128],
                            qt[b][:,
                                  qh * 1024 + cc * 512:qh * 1024 + (cc + 1) * 512],
                            start=True,
                            stop=True,
                        )
                    et_t = etp.tile([128, 1024], f16, name="et")
                    nc.scalar.activation(
                        out=et_t[:, 0:nq * 512],
                        in_=ring_t[:, 0:nq * 512],
                        func=mybir.ActivationFunctionType.Exp,
                        scale=0.125,
                    )
                    pend.append(ev_mm(ktile, et_t))
                    tick()
                pend.append(writeout())

            for b in range(B):
                for hh in range(2):
                    for qh in range(2):
                        unit(b, hh, qh)
            for p in pend:
                p()
            if not done[0]:
                pull_due(force=True)

    nc.compile()
    return nc


def _make_in_maps(x, L, Wq, Wk, Wv):
    x = np.asarray(x, dtype=np.float32)
    L = np.asarray(L)
    ident = np.eye(128, dtype=np.float16)
    xt = np.empty((B, 2, 128, 8, 1024), dtype=np.float16)
    for b in range(B):
        smask = (np.arange(S) < int(L[b])).astype(np.float32)
        xb = (x[b].T * smask[None, :]).reshape(8, 128, 2, 1024).astype(np.float16)
        xt[b] = xb.transpose(2, 1, 0, 3)
    in_maps = []
    for core in range(NCORES):
        m = {"xt": xt, "ident": ident}
        for nm, W in (("wq", Wq), ("wk", Wk), ("wv", Wv)):
            ws = np.asarray(W, dtype=np.float32)[core * 128:(core + 1) * 128, :].T
            m[nm] = np.ascontiguousarray(
                ws.reshape(8, 128, 128).transpose(1, 0, 2), dtype=np.float16
            )
        in_maps.append(m)
    return in_maps


def _postprocess(results, L, bv, Wo, bo):
    L = np.asarray(L)
    KTN, QCH, _, _ = _bounds(L)
    pooled = np.zeros((B, HDIM), dtype=np.float32)
    for core in range(NCORES):
        ev = np.asarray(results[core]["ev"])  # [B, 2, 2, 65, 1024]
        for b in range(B):
            Lb = int(L[b])
            for hh in range(2):
                cols = []
                for qh in range(2):
                    nq = QCH[b][qh]
                    if nq:
                        cols.append(ev[b, hh, qh][:, :nq * 512])
                flat = np.concatenate(cols, axis=1)
                ncols = flat.shape[1]
                if hh == 0:
                    dims, den = flat[0:64], flat[64]
                else:
                    den, dims = flat[0], flat[1:65]
                den_true = den - np.float32(KTN[b] * 128 - Lb)
                valid = np.arange(ncols) < Lb
                r = np.where(valid, 1.0 / (Lb * den_true), 0.0).astype(np.float32)
                g = core * 2 + hh
                pooled[b, g * 64:(g + 1) * 64] = dims @ r
    pooled = pooled + np.asarray(bv, dtype=np.float32)[None, :]
    out = pooled @ np.asarray(Wo, dtype=np.float32).T + np.asarray(bo, np.float32)
    return out.astype(np.float32)


_RUN_KWARGS = {}


def kernel(x, L, Wq, Wk, Wv, bv, Wo, bo):
    from concourse.bass_utils import run_bass_kernel_spmd

    nc = _build_program(np.asarray(L))
    in_maps = _make_in_maps(x, L, Wq, Wk, Wv)
    res = run_bass_kernel_spmd(nc, in_maps, list(range(NCORES)), **_RUN_KWARGS)
    kernel.last_results = res
    return _postprocess(res.results, L, bv, Wo, bo)
